# revision 30
# baseline (speedup 1.0000x reference)
"""DNC single-step forward on 8 Trainium2 NeuronCores (Bass/Tile) — v2.

Data-parallel over batch (B=256 -> 32/core); params replicated.
SINGLE collective: one AllGather of per-core write-addressing factors
(+ alloc/link-sum shards). The (N,D) erase/add matrices are recomputed
redundantly on every core from the gathered rank-factors, which removes
the 1MB AllReduce and the alloc AllGather of the previous design: the
only cross-core serialization left is the once-per-execution CC barrier.

Math restructuring (validated vs reference in numpy at ~3e-3):
  - h0=c0=0  =>  W_hh and the forget gate are dead.
  - prev_rw uniform 1/N  =>  read_flat = tiled col-mean(memory); the
    backward/forward einsums collapse to (1/N)*col/row-sums of link_new.
  - the lu (ww^T ww) link update contributes ~1e-7 rel to the output
    through those sums -> dropped entirely.
  - write_w[b,:] = a_sc[b]*cwexp[b,:] + b_half[b]*alloc[:], so
    erase_mat = cwexp^T @ (a_sc*ev) + alloc outer (b_half . ev); the
    (B,N) write weights are never materialized.
  - allocation weights sort-free: alloc[i] = u[i]*exp(sum_{u_k<u_i} ln(1-u_k)).
"""
import sys

sys.path.insert(0, '/opt/trn_rl_repo')

import numpy as np
import ml_dtypes
import concourse.bass as bass
import concourse.bacc as bacc
import concourse.tile as tile
from concourse import mybir
from concourse.bass_utils import run_bass_kernel_spmd
from concourse.masks import make_identity

AF = mybir.ActivationFunctionType
ALU = mybir.AluOpType
F32 = mybir.dt.float32
BF16 = mybir.dt.bfloat16

B, N, D, R, H, I = 256, 2048, 128, 4, 1024, 1024
CI = I + R * D          # 1536
IF = 787
M = 8                   # cores
BL = B // M             # 32 batch rows per core
NT = N // 128           # 16 n-tiles
KC = CI // 128          # 12 k-tiles of cin
KH = H // 128           # 8 k-tiles of h

# AllGather payload sections (bf16 elements, per core)
P_CW = 0                       # cwexp           [32, 2048] (p-major)
P_EV = P_CW + BL * N           # ev = sig(e)*wg  [32, 128]
P_AV = P_EV + BL * D           # av = wv*wg      [32, 128]
P_C4 = P_AV + BL * D           # [a_sc, b_half, 0, 0] [32, 4]
P_AL = P_C4 + BL * 4           # alloc shard     [(t p)] 256
P_RS = P_AL + 256              # link rowsum shard [(t p)] 256
P_CS = P_RS + 256              # link colsum partial [2048]
PAY = P_CS + N

DEBUG = False


def build_nc():
    nc = bacc.Bacc("TRN2", target_bir_lowering=False, num_devices=M)
    dt = F32
    # ---- inputs (host-prepared layouts) ----
    cinx = nc.declare_dram_parameter("cinx", [128, KH * BL], BF16, isOutput=False)
    w3r = nc.declare_dram_parameter("w3r", [128, KC * 3072], BF16, isOutput=False)
    b3row = nc.declare_dram_parameter("b3row", [1, 3072], BF16, isOutput=False)
    wifr = nc.declare_dram_parameter("wifr", [128, KH * IF], BF16, isOutput=False)
    bifr = nc.declare_dram_parameter("bifr", [1, IF], BF16, isOutput=False)
    woutr = nc.declare_dram_parameter("woutr", [128, KC * 1024], BF16, isOutput=False)
    boutr = nc.declare_dram_parameter("boutr", [1, 1024], BF16, isOutput=False)
    memA = nc.declare_dram_parameter("memA", [128, N], dt, isOutput=False)
    lnk = nc.declare_dram_parameter("lnk", [128, 2 * N], BF16, isOutput=False)
    usg = nc.declare_dram_parameter("usg", [1, N], dt, isOutput=False)
    ucols = nc.declare_dram_parameter("ucols", [128, 2], dt, isOutput=False)

    outF = nc.declare_dram_parameter("outF", [BL, 1024], dt, isOutput=True)
    if DEBUG:
        d_h = nc.declare_dram_parameter("d_h", [BL, 1024], dt, isOutput=True)
        d_itf = nc.declare_dram_parameter("d_itf", [BL, IF], dt, isOutput=True)
        d_cw = nc.declare_dram_parameter("d_cw", [BL, N], dt, isOutput=True)
        d_mnew = nc.declare_dram_parameter("d_mnew", [128, N], dt, isOutput=True)
        d_nrw = nc.declare_dram_parameter("d_nrw", [128, N], dt, isOutput=True)
        d_oacc = nc.declare_dram_parameter("d_oacc", [BL, 1024], dt, isOutput=True)
        d_bw = nc.declare_dram_parameter("d_bw", [2, N], dt, isOutput=True)

    from contextlib import ExitStack
    with tile.TileContext(nc) as tc, ExitStack() as es:
        cons = es.enter_context(tc.tile_pool(name="cons", bufs=1))
        wk = es.enter_context(tc.tile_pool(name="wk", bufs=1))
        wstr = es.enter_context(tc.tile_pool(name="wstr", bufs=4))
        dpool = es.enter_context(tc.tile_pool(name="dram", bufs=1, space="DRAM"))

        ag_in = dpool.tile([PAY], BF16)
        ag_out = dpool.tile([M, PAY], BF16, addr_space="Shared")

        ident = cons.tile([128, 128], dt)
        make_identity(nc, ident)
        ones_col = cons.tile([128, 1], dt)
        nc.vector.memset(ones_col, 1.0)
        ones_row = cons.tile([1, 128], dt)
        nc.vector.memset(ones_row, 1.0)
        ones32 = cons.tile([128, BL], dt)
        nc.vector.memset(ones32, 1.0)

        # ---- bulk input DMAs (early; spread across queues) ----
        mem_sb = cons.tile([128, N], dt)
        nc.sync.dma_start(out=mem_sb, in_=memA[:, :])
        mem3 = mem_sb.rearrange("p (t d) -> p t d", d=128)
        cin = wk.tile([128, KH, BL], BF16)
        nc.sync.dma_start(out=cin,
                          in_=cinx[:, :].rearrange("p (k b) -> p k b", b=BL))
        wif_t = []
        for k in range(KH):
            wf_k = cons.tile([128, IF], BF16, name=f"wif{k}")
            nc.sync.dma_start(out=wf_k, in_=wifr[:, k * IF:(k + 1) * IF])
            wif_t.append(wf_k)
        woutsb = cons.tile([128, KC * 1024], BF16)
        for k in range(KC):
            nc.sync.dma_start(out=woutsb[:, k * 1024:(k + 1) * 1024],
                              in_=woutr[:, k * 1024:(k + 1) * 1024])
        lt2 = cons.tile([128, 2, N], BF16)
        for i in range(2):
            nc.sync.dma_start(out=lt2[:, i, :], in_=lnk[:, i * N:(i + 1) * N])
        ones_cb = cons.tile([128, 1], BF16)
        nc.vector.memset(ones_cb, 1.0)
        b3bc = cons.tile([BL, 3072], BF16)
        nc.sync.dma_start(out=b3bc, in_=b3row[0:1, :].partition_broadcast(BL))
        bif_b = cons.tile([BL, IF], BF16)
        nc.sync.dma_start(out=bif_b, in_=bifr[0:1, :].partition_broadcast(BL))
        bout_bc = cons.tile([BL, 1024], BF16)
        nc.sync.dma_start(out=bout_bc,
                          in_=boutr[0:1, :].partition_broadcast(BL))
        usg_b = wk.tile([128, N], dt, tag="big4")
        nc.sync.dma_start(out=usg_b, in_=usg[0:1, :].partition_broadcast(128))
        uc_sb = cons.tile([128, 2], dt)
        nc.sync.dma_start(out=uc_sb, in_=ucols[:, :])

        hT = cons.tile([128, KH, BL], BF16)
        memnT = cons.tile([128, N], dt)
        csrow = wk.tile([1, N], BF16)
        al2 = wk.tile([128, 2], BF16)
        rs2 = wk.tile([128, 2], BF16)

        # ======== phase 1 (psG psum scope): gates + transposes + colsums ====
        with tc.tile_pool(name="psG", bufs=1, space="PSUM") as psG:
            # col-mean of memory -> cmean stationary [128(d), 32(b)]
            ps_mc = psG.tile([128, 128], dt, tag="tr", bufs=2)
            for t in range(NT):
                nc.tensor.matmul(ps_mc[:, 0:1], lhsT=mem3[:, t, :],
                                 rhs=ones_col, start=(t == 0),
                                 stop=(t == NT - 1))
            mean_col = wk.tile([128, 1], dt)
            nc.scalar.activation(out=mean_col, in_=ps_mc[:, 0:1], func=AF.Copy,
                                 scale=1.0 / N)
            cmean = wk.tile([128, BL], BF16)
            nc.scalar.activation(out=cmean, in_=ones32, func=AF.Copy,
                                 scale=mean_col)

            # LSTM gates, transposed: out [32(b), 3072(j)] in 6 psum banks
            ps_g = psG.tile([BL, 3072], dt, tag="g")
            for k in range(KC):
                w3k = wstr.tile([128, 3072], BF16, tag="w3k")
                for q in range(4):
                    nc.sync.dma_start(
                        out=w3k[:, q * 768:(q + 1) * 768],
                        in_=w3r[:, k * 3072 + q * 768:k * 3072 + (q + 1) * 768])
                lhs_k = cin[:, k, :] if k < KH else cmean
                for g6 in range(6):
                    nc.tensor.matmul(
                        ps_g[:, g6 * 512:(g6 + 1) * 512], lhsT=lhs_k,
                        rhs=w3k[:, g6 * 512:(g6 + 1) * 512],
                        start=(k == 0), stop=(k == KC - 1))
            graw = wk.tile([BL, 3072], dt)
            nc.vector.tensor_tensor(out=graw, in0=ps_g, in1=b3bc, op=ALU.add)
            nc.scalar.activation(out=graw[:, 0:1024], in_=graw[:, 0:1024],
                                 func=AF.Sigmoid)
            nc.scalar.activation(out=graw[:, 2048:3072], in_=graw[:, 2048:3072],
                                 func=AF.Sigmoid)
            nc.scalar.activation(out=graw[:, 1024:2048], in_=graw[:, 1024:2048],
                                 func=AF.Tanh)
            hrow = wk.tile([BL, 1024], dt)
            nc.vector.tensor_mul(out=hrow, in0=graw[:, 0:1024],
                                 in1=graw[:, 1024:2048])
            nc.scalar.activation(out=hrow, in_=hrow, func=AF.Tanh)
            nc.vector.tensor_mul(out=hrow, in0=hrow, in1=graw[:, 2048:3072])
            for t in range(KH):
                ps_t = psG.tile([128, 128], dt, tag="tr", bufs=2)
                nc.tensor.transpose(ps_t[:, 0:BL],
                                    hrow[:, t * 128:(t + 1) * 128],
                                    ident[0:BL, 0:BL])
                nc.vector.tensor_copy(out=hT[:, t, :], in_=ps_t[:, 0:BL])

            # normalized memory rows, transposed -> memnT [128(d), N]
            sqm3 = wk.tile([128, NT, 128], dt, tag="big1")
            nc.vector.tensor_mul(out=sqm3, in0=mem3, in1=mem3)
            mn16 = wk.tile([128, NT], dt)
            nc.vector.tensor_reduce(out=mn16, in_=sqm3,
                                    axis=mybir.AxisListType.X, op=ALU.add)
            nc.scalar.activation(out=mn16, in_=mn16, func=AF.Sqrt)
            nc.vector.tensor_scalar(out=mn16, in0=mn16, scalar1=1e-12,
                                    scalar2=None, op0=ALU.max)
            nc.vector.reciprocal(out=mn16, in_=mn16)
            nc.vector.tensor_tensor(
                out=sqm3, in0=mem3,
                in1=mn16.unsqueeze(2).broadcast_to([128, NT, 128]),
                op=ALU.mult)
            for t in range(NT):
                ps_t = psG.tile([128, 128], dt, tag="tr", bufs=2)
                nc.tensor.transpose(ps_t, sqm3[:, t, :], ident)
                nc.vector.tensor_copy(out=memnT[:, t * 128:(t + 1) * 128],
                                      in_=ps_t)

            # link colsum partial (this core's 256 rows), via ones matmuls
            for ch in range(4):
                ps_cs = psG.tile([1, 512], dt, tag="tr", bufs=2)
                for i in range(2):
                    nc.tensor.matmul(ps_cs, lhsT=ones_cb,
                                     rhs=lt2[:, i, ch * 512:(ch + 1) * 512],
                                     start=(i == 0), stop=(i == 1))
                nc.scalar.copy(out=csrow[:, ch * 512:(ch + 1) * 512],
                               in_=ps_cs)

        # ---- link rowsum shard + allocation shard (DVE/scalar only) ----
        for i in range(2):
            rcol = wk.tile([128, 1], dt, tag="rcol", bufs=2)
            nc.vector.tensor_reduce(out=rcol, in_=lt2[:, i, :],
                                    axis=mybir.AxisListType.X, op=ALU.add)
            nc.vector.tensor_copy(out=rs2[:, i:i + 1], in_=rcol)
        L_b = wk.tile([128, N], dt, tag="big2")
        nc.scalar.activation(out=L_b, in_=usg_b, func=AF.Ln, bias=1.0,
                             scale=-1.0)
        for i in range(2):
            u_col = uc_sb[:, i:i + 1]
            step = wk.tile([128, N], dt, tag="big3")
            nc.vector.tensor_scalar(out=step, in0=usg_b, scalar1=u_col,
                                    scalar2=None, op0=ALU.is_lt)
            nc.vector.tensor_tensor(out=step, in0=step, in1=L_b, op=ALU.mult)
            a_col = wk.tile([128, 1], dt, tag="acol", bufs=2)
            nc.vector.tensor_reduce(out=a_col, in_=step,
                                    axis=mybir.AxisListType.X, op=ALU.add)
            nc.scalar.activation(out=a_col, in_=a_col, func=AF.Exp)
            nc.vector.tensor_mul(out=a_col, in0=a_col, in1=u_col)
            nc.vector.tensor_copy(out=al2[:, i:i + 1], in_=a_col)

        # ======== phase 2 (ppb/ppt/pp1 psum scope): rest of kernel ========
        ppb = es.enter_context(tc.tile_pool(name="ppb", bufs=1, space="PSUM"))
        ppt = es.enter_context(tc.tile_pool(name="ppt", bufs=2, space="PSUM"))
        pp1 = es.enter_context(tc.tile_pool(name="pp1", bufs=1, space="PSUM"))

        # interface vector for own 32 rows
        ps_itf = ppb.tile([BL, IF], dt, tag="big")
        for k in range(KH):
            nc.tensor.matmul(ps_itf[:, 0:512], lhsT=hT[:, k, :],
                             rhs=wif_t[k][:, 0:512], start=(k == 0),
                             stop=(k == KH - 1))
            nc.tensor.matmul(ps_itf[:, 512:IF], lhsT=hT[:, k, :],
                             rhs=wif_t[k][:, 512:IF], start=(k == 0),
                             stop=(k == KH - 1))
        itf = wk.tile([BL, IF], dt)
        nc.vector.tensor_tensor(out=itf, in0=ps_itf, in1=bif_b, op=ALU.add)

        # output-projection h-part, accumulated now (off critical path)
        ps_fh = pp1.tile([BL, 1024], dt, tag="p1")
        for k in range(KH):
            for hf in range(2):
                nc.tensor.matmul(
                    ps_fh[:, hf * 512:(hf + 1) * 512], lhsT=hT[:, k, :],
                    rhs=woutsb[:, k * 1024 + hf * 512:k * 1024 + (hf + 1) * 512],
                    start=(k == 0), stop=(k == KH - 1))
        outacc = cons.tile([BL, 1024], dt)
        nc.vector.tensor_copy(out=outacc, in_=ps_fh)

        # ---- write addressing factors ----
        wv = itf[:, 0:128]
        ersig = wk.tile([BL, 128], dt)
        nc.scalar.activation(out=ersig, in_=itf[:, 128:256], func=AF.Sigmoid)
        wgag = wk.tile([BL, 2], dt)
        nc.scalar.activation(out=wgag, in_=itf[:, 256:258], func=AF.Sigmoid)
        wg = wgag[:, 0:1]
        agt = wgag[:, 1:2]
        expm = wk.tile([BL, 12], dt)
        nc.scalar.activation(out=expm, in_=itf[:, 259:271], func=AF.Exp)
        msum = wk.tile([BL, 4], dt)
        nc.vector.tensor_reduce(out=msum,
                                in_=expm.rearrange("p (r k) -> p r k", k=3),
                                axis=mybir.AxisListType.X, op=ALU.add)
        minv = wk.tile([BL, 4], dt)
        nc.vector.reciprocal(out=minv, in_=msum)
        sc16 = wk.tile([BL, 16], dt)   # [rstr | m0 | m1 | m2]
        nc.scalar.activation(out=sc16[:, 0:4], in_=itf[:, 271:275],
                             func=AF.Exp)
        nc.scalar.activation(out=sc16[:, 0:4], in_=sc16[:, 0:4],
                             func=AF.Ln, bias=1.0)
        em3 = expm.rearrange("p (r k) -> p r k", k=3)
        for kk in range(3):
            nc.vector.tensor_mul(out=sc16[:, 4 + 4 * kk:8 + 4 * kk],
                                 in0=em3[:, :, kk], in1=minv)
        ps_t16 = ppt.tile([16, BL], dt, tag="tr")
        nc.tensor.transpose(ps_t16, sc16, ident[0:BL, 0:BL])
        t16 = wk.tile([16, BL], dt)
        nc.vector.tensor_copy(out=t16, in_=ps_t16)
        cols4 = wk.tile([128, 4], dt)  # [str | m0 | m1 | m2] as rb-columns
        for q in range(4):
            nc.sync.dma_start(out=cols4[:, q:q + 1],
                              in_=t16[4 * q:4 * q + 4, :])
        str_col = cols4[:, 0:1]
        m0_col = cols4[:, 1:2]

        ev_bf = wk.tile([BL, 128], BF16)
        nc.vector.tensor_scalar(out=ev_bf, in0=ersig, scalar1=wg, scalar2=None,
                                op0=ALU.mult)
        av_bf = wk.tile([BL, 128], BF16)
        nc.vector.tensor_scalar(out=av_bf, in0=wv, scalar1=wg, scalar2=None,
                                op0=ALU.mult)

        sq = wk.tile([BL, 128], dt, tag="sq")
        nrm = wk.tile([BL, 1], dt, tag="nrm")
        nc.scalar.activation(out=sq, in_=wv, func=AF.Square, accum_out=nrm)
        nc.scalar.activation(out=nrm, in_=nrm, func=AF.Sqrt)
        nc.vector.tensor_scalar(out=nrm, in0=nrm, scalar1=1e-12, scalar2=None,
                                op0=ALU.max)
        nc.vector.reciprocal(out=nrm, in_=nrm)
        nwv = wk.tile([BL, 128], dt)
        nc.vector.tensor_scalar(out=nwv, in0=wv, scalar1=nrm, scalar2=None,
                                op0=ALU.mult)
        ps_nwvT = ppt.tile([128, BL], dt, tag="tr")
        nc.tensor.transpose(ps_nwvT, nwv, ident[0:BL, 0:BL])
        nwvT = wk.tile([128, BL], dt)
        nc.vector.tensor_copy(out=nwvT, in_=ps_nwvT)

        ps_cw = ppb.tile([BL, N], dt, tag="big")
        for ch in range(4):
            nc.tensor.matmul(ps_cw[:, ch * 512:(ch + 1) * 512], lhsT=nwvT,
                             rhs=memnT[:, ch * 512:(ch + 1) * 512],
                             start=True, stop=True)
        cwexp_bf = wk.tile([BL, N], BF16)
        den = wk.tile([BL, 1], dt)
        nc.scalar.activation(out=cwexp_bf, in_=ps_cw, func=AF.Exp,
                             accum_out=den)
        nc.vector.reciprocal(out=den, in_=den)
        c4 = wk.tile([BL, 4], BF16)
        a_sc = wk.tile([BL, 1], dt)
        nc.vector.tensor_mul(out=a_sc, in0=wg, in1=den)
        nc.vector.tensor_scalar(out=c4[:, 0:1], in0=a_sc, scalar1=0.5,
                                scalar2=None, op0=ALU.mult)
        b_half = wk.tile([BL, 1], dt)
        nc.vector.tensor_mul(out=b_half, in0=wg, in1=agt)
        nc.vector.tensor_scalar(out=c4[:, 1:2], in0=b_half, scalar1=0.5,
                                scalar2=None, op0=ALU.mult)

        # ---- payload stores + the single AllGather ----
        nc.sync.dma_start(
            out=ag_in[P_CW:P_CW + BL * N].rearrange("(p f) -> p f", p=BL),
            in_=cwexp_bf)
        nc.sync.dma_start(
            out=ag_in[P_EV:P_EV + BL * D].rearrange("(p f) -> p f", p=BL),
            in_=ev_bf)
        nc.sync.dma_start(
            out=ag_in[P_AV:P_AV + BL * D].rearrange("(p f) -> p f", p=BL),
            in_=av_bf)
        nc.sync.dma_start(
            out=ag_in[P_C4:P_C4 + BL * 4].rearrange("(p f) -> p f", p=BL),
            in_=c4)
        nc.sync.dma_start(
            out=ag_in[P_AL:P_AL + 256].rearrange("(t p) -> p t", p=128),
            in_=al2)
        nc.sync.dma_start(
            out=ag_in[P_RS:P_RS + 256].rearrange("(t p) -> p t", p=128),
            in_=rs2)
        nc.sync.dma_start(out=ag_in[P_CS:P_CS + N], in_=csrow)

        nc.gpsimd.collective_compute(
            "AllGather", ALU.bypass, replica_groups=[list(range(M))],
            ins=[ag_in[:]], outs=[ag_out.flatten()])

        # ---- post-AG loads ----
        cwf = []
        for bc in range(2):
            t_cw = wk.tile([128, N], BF16, name=f"cwf{bc}")
            for r in range(4):
                rk = bc * 4 + r
                nc.sync.dma_start(
                    out=t_cw[r * BL:(r + 1) * BL, :],
                    in_=ag_out[rk, P_CW:P_CW + BL * N]
                    .rearrange("(p f) -> p f", p=BL))
            cwf.append(t_cw)
        evf = wk.tile([128, 2, 128], BF16)   # [b128, bc, d]
        avf = wk.tile([128, 2, 128], BF16)
        c4f = wk.tile([128, 2, 4], BF16)
        for bc in range(2):
            for r in range(4):
                rk = bc * 4 + r
                sl = slice(r * BL, (r + 1) * BL)
                nc.sync.dma_start(
                    out=evf[sl, bc, :],
                    in_=ag_out[rk, P_EV:P_EV + BL * D]
                    .rearrange("(p f) -> p f", p=BL))
                nc.sync.dma_start(
                    out=avf[sl, bc, :],
                    in_=ag_out[rk, P_AV:P_AV + BL * D]
                    .rearrange("(p f) -> p f", p=BL))
                nc.sync.dma_start(
                    out=c4f[sl, bc, :],
                    in_=ag_out[rk, P_C4:P_C4 + BL * 4]
                    .rearrange("(p f) -> p f", p=BL))
        alc16 = wk.tile([128, NT], BF16)
        rsrow = wk.tile([1, N], BF16)
        for r in range(M):
            nc.sync.dma_start(
                out=alc16[:, 2 * r:2 * r + 2],
                in_=ag_out[r, P_AL:P_AL + 256].rearrange("(t p) -> p t", p=128))
            nc.sync.dma_start(out=rsrow[:, r * 256:(r + 1) * 256],
                              in_=ag_out[r, P_RS:P_RS + 256])
        cs8 = wk.tile([M, N], BF16)
        nc.sync.dma_start(out=cs8, in_=ag_out[:, P_CS:P_CS + N])

        # scaled rhs [a_sc*ev | a_sc*av] per batch-chunk + b_half columns
        asc_col = wk.tile([128, 2], dt)
        nc.vector.tensor_copy(out=asc_col, in_=c4f[:, :, 0])
        bh_col = wk.tile([128, 2], BF16)
        nc.vector.tensor_copy(out=bh_col, in_=c4f[:, :, 1])
        rhs_eva = wk.tile([128, 2, 256], BF16)
        for bc in range(2):
            nc.vector.tensor_scalar(out=rhs_eva[:, bc, 0:128],
                                    in0=evf[:, bc, :],
                                    scalar1=asc_col[:, bc:bc + 1],
                                    scalar2=None, op0=ALU.mult)
            nc.vector.tensor_scalar(out=rhs_eva[:, bc, 128:256],
                                    in0=avf[:, bc, :],
                                    scalar1=asc_col[:, bc:bc + 1],
                                    scalar2=None, op0=ALU.mult)

        # evb/avb rows = b_half . [ev|av], broadcast to 128 partitions
        evab_r = wk.tile([1, 256], dt)
        ps_evb = ppt.tile([1, 128], dt, tag="tr")
        for bc in range(2):
            nc.tensor.matmul(ps_evb, lhsT=bh_col[:, bc:bc + 1],
                             rhs=evf[:, bc, :], start=(bc == 0),
                             stop=(bc == 1))
        nc.vector.tensor_copy(out=evab_r[:, 0:128], in_=ps_evb)
        ps_avb = ppt.tile([1, 128], dt, tag="tr")
        for bc in range(2):
            nc.tensor.matmul(ps_avb, lhsT=bh_col[:, bc:bc + 1],
                             rhs=avf[:, bc, :], start=(bc == 0),
                             stop=(bc == 1))
        nc.vector.tensor_copy(out=evab_r[:, 128:256], in_=ps_avb)
        ps_ebc = ppt.tile([128, 256], dt, tag="tr")
        nc.tensor.matmul(ps_ebc, lhsT=ones_row, rhs=evab_r, start=True,
                         stop=True)
        evab_bc = wk.tile([128, 256], dt)
        nc.vector.tensor_copy(out=evab_bc, in_=ps_ebc)

        # bw/fw rows: 0.9/N * [colsum(link), rowsum(link)]
        bwrow = wk.tile([1, N], dt)
        for ch in range(4):
            ps_cs2 = ppt.tile([1, 512], dt, tag="tr")
            nc.tensor.matmul(ps_cs2, lhsT=ones_cb[0:8, :],
                             rhs=cs8[:, ch * 512:(ch + 1) * 512],
                             start=True, stop=True)
            nc.vector.tensor_scalar(out=bwrow[:, ch * 512:(ch + 1) * 512],
                                    in0=ps_cs2, scalar1=0.9 / N,
                                    scalar2=None, op0=ALU.mult)
        fwrow = wk.tile([1, N], dt)
        nc.vector.tensor_scalar(out=fwrow, in0=rsrow, scalar1=0.9 / N,
                                scalar2=None, op0=ALU.mult)

        # ---- erase/add matmuls + mnew, pipelined per n-tile ----
        mnew = wk.tile([128, N], dt, tag="big1")
        mnew3 = mnew.rearrange("p (t d) -> p t d", d=128)
        for t in range(NT):
            ps_ea = ppt.tile([128, 256], dt, tag="tr")
            for bc in range(2):
                nc.tensor.matmul(ps_ea, lhsT=cwf[bc][:, t * 128:(t + 1) * 128],
                                 rhs=rhs_eva[:, bc, :], start=(bc == 0),
                                 stop=(bc == 1))
            al_t = alc16[:, t:t + 1]
            e1 = wk.tile([128, 256], dt, tag="e1", bufs=2)
            nc.vector.scalar_tensor_tensor(out=e1, in0=evab_bc, scalar=al_t,
                                           in1=ps_ea, op0=ALU.mult,
                                           op1=ALU.add)
            f1 = wk.tile([128, 128], dt, tag="f1", bufs=2)
            nc.vector.tensor_scalar(out=f1, in0=e1[:, 0:128],
                                    scalar1=-1.0 / B, scalar2=1.0,
                                    op0=ALU.mult, op1=ALU.add)
            nc.vector.tensor_mul(out=f1, in0=f1, in1=mem3[:, t, :])
            nc.vector.scalar_tensor_tensor(out=mnew3[:, t, :],
                                           in0=e1[:, 128:256],
                                           scalar=1.0 / B, in1=f1,
                                           op0=ALU.mult, op1=ALU.add)

        # mnew row norms + normalized transpose
        sqf = wk.tile([128, NT, 128], dt, tag="big2")
        nc.vector.tensor_mul(out=sqf, in0=mnew3, in1=mnew3)
        nrm16 = wk.tile([128, NT], dt)
        nc.vector.tensor_reduce(out=nrm16, in_=sqf, axis=mybir.AxisListType.X,
                                op=ALU.add)
        nc.scalar.activation(out=nrm16, in_=nrm16, func=AF.Sqrt)
        nc.vector.tensor_scalar(out=nrm16, in0=nrm16, scalar1=1e-12,
                                scalar2=None, op0=ALU.max)
        nc.vector.reciprocal(out=nrm16, in_=nrm16)
        nmn = wk.tile([128, NT, 128], dt, tag="big3")
        nc.vector.tensor_tensor(
            out=nmn, in0=mnew3,
            in1=nrm16.unsqueeze(2).broadcast_to([128, NT, 128]), op=ALU.mult)
        mnewT = memnT  # reuse buffer: memnT dead after cw sim
        for t in range(NT):
            ps_t = ppt.tile([128, 128], dt, tag="tr")
            nc.tensor.transpose(ps_t, nmn[:, t, :], ident)
            nc.vector.tensor_copy(out=mnewT[:, t * 128:(t + 1) * 128],
                                  in_=ps_t)

        # ---- read addressing (own 32 rows; rb = r*32+b on partitions) ----
        nkT = wk.tile([128, 128], dt)
        rk3 = itf[:, 275:787].rearrange("p (r d) -> p r d", d=128)
        sqk3 = wk.tile([BL, R, 128], dt)
        nc.vector.tensor_mul(out=sqk3, in0=rk3, in1=rk3)
        nrk4 = wk.tile([BL, R], dt)
        nc.vector.tensor_reduce(out=nrk4, in_=sqk3, axis=mybir.AxisListType.X,
                                op=ALU.add)
        nc.scalar.activation(out=nrk4, in_=nrk4, func=AF.Sqrt)
        nc.vector.tensor_scalar(out=nrk4, in0=nrk4, scalar1=1e-12,
                                scalar2=None, op0=ALU.max)
        nc.vector.reciprocal(out=nrk4, in_=nrk4)
        nc.vector.tensor_tensor(
            out=sqk3, in0=rk3,
            in1=nrk4.unsqueeze(2).broadcast_to([BL, R, 128]), op=ALU.mult)
        for r in range(R):
            ps_k = ppt.tile([128, BL], dt, tag="tr")
            nc.tensor.transpose(ps_k, sqk3[:, r, :], ident[0:BL, 0:BL])
            nc.vector.tensor_copy(out=nkT[:, r * BL:(r + 1) * BL], in_=ps_k)

        ps_sim = ppb.tile([128, N], dt, tag="big")
        for ch in range(4):
            nc.tensor.matmul(ps_sim[:, ch * 512:(ch + 1) * 512], lhsT=nkT,
                             rhs=mnewT[:, ch * 512:(ch + 1) * 512],
                             start=True, stop=True)
        esim = wk.tile([128, N], dt, tag="big2")
        dsum = wk.tile([128, 1], dt)
        nc.scalar.activation(out=esim, in_=ps_sim, func=AF.Exp, scale=str_col,
                             accum_out=dsum)
        nc.vector.reciprocal(out=dsum, in_=dsum)
        c0 = wk.tile([128, 1], dt)
        nc.vector.tensor_mul(out=c0, in0=m0_col, in1=dsum)
        ps_m1 = ppt.tile([1, 128], dt, tag="tr")
        nc.tensor.transpose(ps_m1, cols4[:, 2:3], ident)
        m1T = wk.tile([1, 128], dt)
        nc.vector.tensor_copy(out=m1T, in_=ps_m1)
        ps_m2 = ppt.tile([1, 128], dt, tag="tr")
        nc.tensor.transpose(ps_m2, cols4[:, 3:4], ident)
        m2T = wk.tile([1, 128], dt)
        nc.vector.tensor_copy(out=m2T, in_=ps_m2)
        ps_term = ppb.tile([128, N], dt, tag="big")
        for ch in range(4):
            nc.tensor.matmul(ps_term[:, ch * 512:(ch + 1) * 512], lhsT=m1T,
                             rhs=bwrow[:, ch * 512:(ch + 1) * 512],
                             start=True, stop=False)
            nc.tensor.matmul(ps_term[:, ch * 512:(ch + 1) * 512], lhsT=m2T,
                             rhs=fwrow[:, ch * 512:(ch + 1) * 512],
                             start=False, stop=True)
        nrw = esim
        for ch in range(4):
            nc.vector.scalar_tensor_tensor(
                out=nrw[:, ch * 512:(ch + 1) * 512],
                in0=esim[:, ch * 512:(ch + 1) * 512], scalar=c0,
                in1=ps_term[:, ch * 512:(ch + 1) * 512], op0=ALU.mult,
                op1=ALU.add)

        ps_ro = pp1.tile([128, 128], dt, tag="p1")
        roT = wk.tile([128, 128], BF16)
        for t in range(NT):
            ps_tr = ppt.tile([128, 128], dt, tag="tr")
            nc.tensor.transpose(ps_tr, nrw[:, t * 128:(t + 1) * 128], ident)
            nrwT = wk.tile([128, 128], dt, tag="nrwT", bufs=2)
            nc.vector.tensor_copy(out=nrwT, in_=ps_tr)
            nc.tensor.matmul(ps_ro, lhsT=mnew[:, t * 128:(t + 1) * 128],
                             rhs=nrwT, start=(t == 0), stop=(t == NT - 1))
        nc.vector.tensor_copy(out=roT, in_=ps_ro)

        # ---- output projection read-part + writeback ----
        ps_f2 = pp1.tile([BL, 1024], dt, tag="p1")
        for k in range(4):
            for hf in range(2):
                nc.tensor.matmul(
                    ps_f2[:, hf * 512:(hf + 1) * 512],
                    lhsT=roT[:, k * BL:(k + 1) * BL],
                    rhs=woutsb[:, (KH + k) * 1024 + hf * 512:
                               (KH + k) * 1024 + (hf + 1) * 512],
                    start=(k == 0), stop=(k == 3))
        nc.vector.tensor_tensor(out=outacc, in0=ps_f2, in1=outacc, op=ALU.add)
        nc.vector.tensor_tensor(out=outacc, in0=outacc, in1=bout_bc,
                                op=ALU.add)
        nc.sync.dma_start(out=outF[:, :], in_=outacc)

        if DEBUG:
            nc.sync.dma_start(out=d_h[:, :], in_=hrow)
            nc.sync.dma_start(out=d_itf[:, :], in_=itf)
            d_cwf = wk.tile([BL, N], dt)
            nc.vector.tensor_copy(out=d_cwf, in_=cwexp_bf)
            nc.sync.dma_start(out=d_cw[:, :], in_=d_cwf)
            nc.sync.dma_start(out=d_mnew[:, :], in_=mnew)
            nc.sync.dma_start(out=d_nrw[:, :], in_=nrw)
            nc.sync.dma_start(out=d_oacc[:, :], in_=outacc)
            nc.sync.dma_start(out=d_bw[0:1, :], in_=bwrow)
            nc.sync.dma_start(out=d_bw[1:2, :], in_=fwrow)

    nc.finalize()
    return nc


def _prep_inputs(x, memory, usage, link, W_ih, W_hh, b_ih, b_hh, W_if, b_if,
                 W_out, b_out):
    f = np.float32
    x = np.asarray(x, f); memory = np.asarray(memory, f)
    usage = np.asarray(usage, f); link = np.asarray(link, f)
    W_ih = np.asarray(W_ih, f); b_ih = np.asarray(b_ih, f)
    b_hh = np.asarray(b_hh, f); W_if = np.asarray(W_if, f)
    b_if = np.asarray(b_if, f); W_out = np.asarray(W_out, f)
    b_out = np.asarray(b_out, f)

    sel = np.r_[0:1024, 2048:4096]
    W3T = W_ih[sel].T                             # (1536, 3072)
    w3r = np.ascontiguousarray(
        W3T.reshape(KC, 128, 3072).transpose(1, 0, 2)
        .reshape(128, KC * 3072).astype(ml_dtypes.bfloat16))
    b3row = np.ascontiguousarray(
        (b_ih + b_hh)[sel].reshape(1, 3072).astype(ml_dtypes.bfloat16))
    wifr = np.ascontiguousarray(
        W_if.T.reshape(KH, 128, IF).transpose(1, 0, 2)
        .reshape(128, KH * IF).astype(ml_dtypes.bfloat16))
    woutr = np.ascontiguousarray(
        W_out.T.reshape(KC, 128, 1024).transpose(1, 0, 2)
        .reshape(128, KC * 1024).astype(ml_dtypes.bfloat16))
    boutr = b_out.reshape(1, 1024)
    memA = np.ascontiguousarray(
        memory.reshape(NT, 128, 128).transpose(1, 0, 2).reshape(128, N))
    bifr = b_if.reshape(1, IF).astype(ml_dtypes.bfloat16)
    boutr = boutr.astype(ml_dtypes.bfloat16)
    usgr = usage.reshape(1, N)

    shared = dict(w3r=w3r, b3row=b3row, wifr=wifr, bifr=bifr, woutr=woutr,
                  boutr=boutr, memA=memA, usg=usgr)
    in_maps = []
    for c in range(M):
        xs = x[c * BL:(c + 1) * BL]               # (32, 1024)
        cinx = np.ascontiguousarray(
            xs.T.reshape(KH, 128, BL).transpose(1, 0, 2)
            .reshape(128, KH * BL).astype(ml_dtypes.bfloat16))
        ls = link[c * 256:(c + 1) * 256]          # (256, 2048)
        lnkm = np.ascontiguousarray(
            ls.reshape(2, 128, N).transpose(1, 0, 2)
            .reshape(128, 2 * N).astype(ml_dtypes.bfloat16))
        ucols = np.ascontiguousarray(
            usage.reshape(NT, 128)[2 * c:2 * c + 2].T)      # (128, 2)
        m = dict(shared)
        m["cinx"] = cinx
        m["lnk"] = lnkm
        m["ucols"] = ucols
        in_maps.append(m)
    return in_maps


def kernel(**inputs):
    nc = build_nc()
    in_maps = _prep_inputs(**inputs)
    res = run_bass_kernel_spmd(nc, in_maps, list(range(M))).results
    return np.concatenate([res[c]["outF"] for c in range(M)],
                          0).astype(np.float32)


# revision 35
# speedup vs baseline: 1.0192x; 1.0192x over previous
"""DNC single-step forward on 8 Trainium2 NeuronCores (Bass/Tile) — v3.

Data-parallel over batch (B=256 -> 32/core); params replicated.
SINGLE collective: one ~30KB/rank AllGather (Mesh) of per-core write
factors [nwv | ev | av | wg,b_half | alloc shard | link rowsum shard |
link colsum partial]. Every core then redundantly recomputes the full
(B,N) content weights and the (N,D) erase/add matrices from the
gathered factors — cheaper than AllReducing the 1MB matrices, and the
only cross-core serialization left is the once-per-execution CC
barrier.

Math restructuring (validated vs reference in numpy at ~3e-3):
  - h0=c0=0  =>  W_hh and the forget gate are dead.
  - prev_rw uniform 1/N  =>  read_flat = tiled col-mean(memory); the
    backward/forward einsums collapse to (1/N)*col/row-sums of link_new.
  - the lu (ww^T ww) link update contributes ~1e-7 rel -> dropped.
  - write_w[b,:] = a_sc[b]*cwexp[b,:] + b_half[b]*alloc[:], so
    erase_mat = cwexp^T @ (a_sc*ev) + alloc outer (b_half . ev); the
    write weights are never materialized.
  - allocation weights sort-free: alloc[i] = u[i]*exp(sum_{u_k<u_i} ln(1-u_k)).
"""
import sys

sys.path.insert(0, '/opt/trn_rl_repo')

import numpy as np
import ml_dtypes
import concourse.bass as bass
import concourse.bacc as bacc
import concourse.tile as tile
from concourse import mybir
from concourse.bass_utils import run_bass_kernel_spmd
from concourse.masks import make_identity

AF = mybir.ActivationFunctionType
ALU = mybir.AluOpType
F32 = mybir.dt.float32
BF16 = mybir.dt.bfloat16

B, N, D, R, H, I = 256, 2048, 128, 4, 1024, 1024
CI = I + R * D          # 1536
IF = 787
M = 8                   # cores
BL = B // M             # 32 batch rows per core
NT = N // 128           # 16 n-tiles
KC = CI // 128          # 12 k-tiles of cin
KH = H // 128           # 8 k-tiles of h

# AllGather payload (bf16 elements, per core)
# X section, row-major [32, 388]: [nwv(128) | ev(128) | av(128) | c4(4)]
XW = 388
P_X = 0
P_AL = P_X + BL * XW           # alloc shard, (i p) flat, 256
P_RS = P_AL + 256              # link rowsum shard, (i p) flat, 256
P_CS = P_RS + 256              # link colsum partial, 2048
PAY = P_CS + N                 # 14976 el = 29952 B/rank -> Mesh

DEBUG = False


def build_nc():
    nc = bacc.Bacc("TRN2", target_bir_lowering=False, num_devices=M)
    dt = F32
    # ---- inputs (host-prepared layouts) ----
    cinx = nc.declare_dram_parameter("cinx", [128, KH * BL], BF16, isOutput=False)
    w3r = nc.declare_dram_parameter("w3r", [128, KC * 3072], BF16, isOutput=False)
    b3row = nc.declare_dram_parameter("b3row", [1, 3072], BF16, isOutput=False)
    wifr = nc.declare_dram_parameter("wifr", [128, KH * IF], BF16, isOutput=False)
    bifr = nc.declare_dram_parameter("bifr", [1, IF], BF16, isOutput=False)
    woutr = nc.declare_dram_parameter("woutr", [128, KC * 1024], BF16, isOutput=False)
    boutr = nc.declare_dram_parameter("boutr", [1, 1024], BF16, isOutput=False)
    memA = nc.declare_dram_parameter("memA", [128, N], dt, isOutput=False)
    lnk = nc.declare_dram_parameter("lnk", [128, 2 * N], BF16, isOutput=False)
    usg = nc.declare_dram_parameter("usg", [1, N], dt, isOutput=False)
    ucols = nc.declare_dram_parameter("ucols", [128, 2], dt, isOutput=False)

    outF = nc.declare_dram_parameter("outF", [BL, 1024], dt, isOutput=True)
    if DEBUG:
        d_h = nc.declare_dram_parameter("d_h", [BL, 1024], dt, isOutput=True)
        d_itf = nc.declare_dram_parameter("d_itf", [BL, IF], dt, isOutput=True)
        d_cw = nc.declare_dram_parameter("d_cw", [128, N], dt, isOutput=True)
        d_mnew = nc.declare_dram_parameter("d_mnew", [128, N], dt, isOutput=True)
        d_nrw = nc.declare_dram_parameter("d_nrw", [128, N], dt, isOutput=True)
        d_bw = nc.declare_dram_parameter("d_bw", [2, N], dt, isOutput=True)

    from contextlib import ExitStack
    with tile.TileContext(nc) as tc, ExitStack() as es:
        cons = es.enter_context(tc.tile_pool(name="cons", bufs=1))
        wk = es.enter_context(tc.tile_pool(name="wk", bufs=1))
        wstr = es.enter_context(tc.tile_pool(name="wstr", bufs=4))
        dpool = es.enter_context(tc.tile_pool(name="dram", bufs=1, space="DRAM"))

        ag_in = dpool.tile([PAY], BF16)
        ag_out = dpool.tile([M, PAY], BF16, addr_space="Shared")

        # ---- bulk input DMAs; w3 stream split across sync+scalar queues ---
        w3k_t = []
        for k in range(KC):
            w3k = wstr.tile([128, 3072], BF16, tag="w3k")
            for q in range(3):
                eng = nc.sync if (k * 3 + q) % 2 == 0 else nc.scalar
                eng.dma_start(
                    out=w3k[:, q * 1024:(q + 1) * 1024],
                    in_=w3r[:, k * 3072 + q * 1024:k * 3072 + (q + 1) * 1024])
            w3k_t.append(w3k)
        cin = wk.tile([128, KH, BL], BF16)
        nc.sync.dma_start(out=cin,
                          in_=cinx[:, :].rearrange("p (k b) -> p k b", b=BL))
        mem_sb = cons.tile([128, N], dt)
        nc.sync.dma_start(out=mem_sb, in_=memA[:, :])
        mem3 = mem_sb.rearrange("p (t d) -> p t d", d=128)
        lt2 = cons.tile([128, 2, N], BF16)
        for i in range(2):
            nc.sync.dma_start(out=lt2[:, i, :], in_=lnk[:, i * N:(i + 1) * N])
        wif_t = []
        for k in range(KH):
            wf_k = cons.tile([128, IF], BF16, name=f"wif{k}")
            eng = nc.sync if k % 2 == 0 else nc.scalar
            eng.dma_start(out=wf_k, in_=wifr[:, k * IF:(k + 1) * IF])
            wif_t.append(wf_k)
        woutsb = cons.tile([128, KC * 1024], BF16)
        for k in range(KC):
            eng = nc.sync if k % 2 == 0 else nc.scalar
            eng.dma_start(out=woutsb[:, k * 1024:(k + 1) * 1024],
                          in_=woutr[:, k * 1024:(k + 1) * 1024])
        b3bc = cons.tile([BL, 3072], BF16)
        nc.sync.dma_start(out=b3bc, in_=b3row[0:1, :].partition_broadcast(BL))
        bif_b = cons.tile([BL, IF], BF16)
        nc.sync.dma_start(out=bif_b, in_=bifr[0:1, :].partition_broadcast(BL))
        bout_bc = cons.tile([BL, 1024], BF16)
        nc.sync.dma_start(out=bout_bc,
                          in_=boutr[0:1, :].partition_broadcast(BL))
        usg_b = wk.tile([128, N], dt, tag="big4")
        nc.sync.dma_start(out=usg_b, in_=usg[0:1, :].partition_broadcast(128))
        uc_sb = cons.tile([128, 2], dt)
        nc.sync.dma_start(out=uc_sb, in_=ucols[:, :])

        ident = cons.tile([128, 128], dt)
        make_identity(nc, ident)
        ident_bf = cons.tile([128, 128], BF16)
        make_identity(nc, ident_bf)
        ones_col = cons.tile([128, 1], dt)
        nc.vector.memset(ones_col, 1.0)
        ones_cb = cons.tile([128, 1], BF16)
        nc.vector.memset(ones_cb, 1.0)
        ones32 = cons.tile([128, BL], dt)
        nc.vector.memset(ones32, 1.0)

        hT = cons.tile([128, KH, BL], BF16)
        memnT = cons.tile([128, N], dt)
        csrow = wk.tile([1, N], BF16)
        al2 = wk.tile([128, 2], BF16)
        rs2 = wk.tile([128, 2], BF16)

        # ======== phase 1 (psG psum scope): gates + transposes + colsums ====
        with tc.tile_pool(name="psG", bufs=1, space="PSUM") as psG:
            # col-mean of memory -> cmean stationary [128(d), 32(b)]
            ps_mc = psG.tile([128, 128], dt, tag="tr", bufs=2)
            for t in range(NT):
                nc.tensor.matmul(ps_mc[:, 0:1], lhsT=mem3[:, t, :],
                                 rhs=ones_col, start=(t == 0),
                                 stop=(t == NT - 1))
            mean_col = wk.tile([128, 1], dt)
            nc.scalar.activation(out=mean_col, in_=ps_mc[:, 0:1], func=AF.Copy,
                                 scale=1.0 / N)
            cmean = wk.tile([128, BL], BF16)
            nc.scalar.activation(out=cmean, in_=ones32, func=AF.Copy,
                                 scale=mean_col)

            # LSTM gates, transposed: out [32(b), 3072(j)] in 6 psum banks
            ps_g = psG.tile([BL, 3072], dt, tag="g")
            for k in range(KC):
                lhs_k = cin[:, k, :] if k < KH else cmean
                for g6 in range(6):
                    nc.tensor.matmul(
                        ps_g[:, g6 * 512:(g6 + 1) * 512], lhsT=lhs_k,
                        rhs=w3k_t[k][:, g6 * 512:(g6 + 1) * 512],
                        start=(k == 0), stop=(k == KC - 1))
            graw = wk.tile([BL, 3072], dt)
            nc.vector.tensor_tensor(out=graw, in0=ps_g, in1=b3bc, op=ALU.add)
            nc.scalar.activation(out=graw[:, 0:1024], in_=graw[:, 0:1024],
                                 func=AF.Sigmoid)
            nc.scalar.activation(out=graw[:, 2048:3072], in_=graw[:, 2048:3072],
                                 func=AF.Sigmoid)
            nc.scalar.activation(out=graw[:, 1024:2048], in_=graw[:, 1024:2048],
                                 func=AF.Tanh)
            hrow = wk.tile([BL, 1024], dt)
            nc.vector.tensor_mul(out=hrow, in0=graw[:, 0:1024],
                                 in1=graw[:, 1024:2048])
            nc.scalar.activation(out=hrow, in_=hrow, func=AF.Tanh)
            nc.vector.tensor_mul(out=hrow, in0=hrow, in1=graw[:, 2048:3072])
            for t in range(KH):
                ps_t = psG.tile([128, 128], dt, tag="tr", bufs=2)
                nc.tensor.transpose(ps_t[:, 0:BL],
                                    hrow[:, t * 128:(t + 1) * 128],
                                    ident[0:BL, 0:BL])
                nc.vector.tensor_copy(out=hT[:, t, :], in_=ps_t[:, 0:BL])

            # normalized memory rows, transposed -> memnT [128(d), N]
            sqm3 = wk.tile([128, NT, 128], dt, tag="big1")
            nc.vector.tensor_mul(out=sqm3, in0=mem3, in1=mem3)
            mn16 = wk.tile([128, NT], dt)
            nc.vector.tensor_reduce(out=mn16, in_=sqm3,
                                    axis=mybir.AxisListType.X, op=ALU.add)
            nc.scalar.activation(out=mn16, in_=mn16, func=AF.Sqrt)
            nc.vector.tensor_scalar(out=mn16, in0=mn16, scalar1=1e-12,
                                    scalar2=None, op0=ALU.max)
            nc.vector.reciprocal(out=mn16, in_=mn16)
            nc.vector.tensor_tensor(
                out=sqm3, in0=mem3,
                in1=mn16.unsqueeze(2).broadcast_to([128, NT, 128]),
                op=ALU.mult)
            for t in range(NT):
                ps_t = psG.tile([128, 128], dt, tag="tr", bufs=2)
                nc.tensor.transpose(ps_t, sqm3[:, t, :], ident)
                nc.vector.tensor_copy(out=memnT[:, t * 128:(t + 1) * 128],
                                      in_=ps_t)

            # link colsum partial (this core's 256 rows), via ones matmuls
            for ch in range(4):
                ps_cs = psG.tile([1, 512], dt, tag="tr", bufs=2)
                for i in range(2):
                    nc.tensor.matmul(ps_cs, lhsT=ones_cb,
                                     rhs=lt2[:, i, ch * 512:(ch + 1) * 512],
                                     start=(i == 0), stop=(i == 1))
                nc.scalar.copy(out=csrow[:, ch * 512:(ch + 1) * 512],
                               in_=ps_cs)

        # ---- link rowsum shard + allocation shard (DVE/scalar only) ----
        for i in range(2):
            rcol = wk.tile([128, 1], dt, tag="rcol", bufs=2)
            nc.vector.tensor_reduce(out=rcol, in_=lt2[:, i, :],
                                    axis=mybir.AxisListType.X, op=ALU.add)
            nc.vector.tensor_copy(out=rs2[:, i:i + 1], in_=rcol)
        L_b = wk.tile([128, N], dt, tag="big2")
        nc.scalar.activation(out=L_b, in_=usg_b, func=AF.Ln, bias=1.0,
                             scale=-1.0)
        for i in range(2):
            u_col = uc_sb[:, i:i + 1]
            step = wk.tile([128, N], dt, tag="big3")
            nc.vector.tensor_scalar(out=step, in0=usg_b, scalar1=u_col,
                                    scalar2=None, op0=ALU.is_lt)
            nc.vector.tensor_tensor(out=step, in0=step, in1=L_b, op=ALU.mult)
            a_col = wk.tile([128, 1], dt, tag="acol", bufs=2)
            nc.vector.tensor_reduce(out=a_col, in_=step,
                                    axis=mybir.AxisListType.X, op=ALU.add)
            nc.scalar.activation(out=a_col, in_=a_col, func=AF.Exp)
            nc.vector.tensor_mul(out=a_col, in0=a_col, in1=u_col)
            nc.vector.tensor_copy(out=al2[:, i:i + 1], in_=a_col)

        # ======== phase 2 (ppb/ppt/pp1 psum scope): rest of kernel ========
        ppb = es.enter_context(tc.tile_pool(name="ppb", bufs=1, space="PSUM"))
        ppt = es.enter_context(tc.tile_pool(name="ppt", bufs=2, space="PSUM"))
        pp1 = es.enter_context(tc.tile_pool(name="pp1", bufs=1, space="PSUM"))

        # interface vector for own 32 rows
        ps_itf = ppb.tile([BL, IF], dt, tag="big")
        for k in range(KH):
            nc.tensor.matmul(ps_itf[:, 0:512], lhsT=hT[:, k, :],
                             rhs=wif_t[k][:, 0:512], start=(k == 0),
                             stop=(k == KH - 1))
            nc.tensor.matmul(ps_itf[:, 512:IF], lhsT=hT[:, k, :],
                             rhs=wif_t[k][:, 512:IF], start=(k == 0),
                             stop=(k == KH - 1))
        itf = wk.tile([BL, IF], dt)
        nc.vector.tensor_tensor(out=itf, in0=ps_itf, in1=bif_b, op=ALU.add)

        # output-projection h-part, accumulated now (off critical path)
        ps_fh = pp1.tile([BL, 1024], dt, tag="p1")
        for k in range(KH):
            for hf in range(2):
                nc.tensor.matmul(
                    ps_fh[:, hf * 512:(hf + 1) * 512], lhsT=hT[:, k, :],
                    rhs=woutsb[:, k * 1024 + hf * 512:k * 1024 + (hf + 1) * 512],
                    start=(k == 0), stop=(k == KH - 1))
        outacc = cons.tile([BL, 1024], dt)
        nc.vector.tensor_copy(out=outacc, in_=ps_fh)

        # ---- write addressing factors -> X payload row [32, 388] ----
        xrow = wk.tile([BL, XW], BF16)
        wv = itf[:, 0:128]
        nc.scalar.activation(out=xrow[:, 128:256], in_=itf[:, 128:256],
                             func=AF.Sigmoid)    # sig(erase), *wg below
        wgag = wk.tile([BL, 2], dt)
        nc.scalar.activation(out=wgag, in_=itf[:, 256:258], func=AF.Sigmoid)
        wg = wgag[:, 0:1]
        agt = wgag[:, 1:2]
        expm = wk.tile([BL, 12], dt)
        nc.scalar.activation(out=expm, in_=itf[:, 259:271], func=AF.Exp)
        msum = wk.tile([BL, 4], dt)
        nc.vector.tensor_reduce(out=msum,
                                in_=expm.rearrange("p (r k) -> p r k", k=3),
                                axis=mybir.AxisListType.X, op=ALU.add)
        minv = wk.tile([BL, 4], dt)
        nc.vector.reciprocal(out=minv, in_=msum)
        sc16 = wk.tile([BL, 16], dt)   # [rstr | m0 | m1 | m2]
        nc.scalar.activation(out=sc16[:, 0:4], in_=itf[:, 271:275],
                             func=AF.Exp)
        nc.scalar.activation(out=sc16[:, 0:4], in_=sc16[:, 0:4],
                             func=AF.Ln, bias=1.0)
        em3 = expm.rearrange("p (r k) -> p r k", k=3)
        for kk in range(3):
            nc.vector.tensor_mul(out=sc16[:, 4 + 4 * kk:8 + 4 * kk],
                                 in0=em3[:, :, kk], in1=minv)
        ps_t16 = ppt.tile([16, BL], dt, tag="tr")
        nc.tensor.transpose(ps_t16, sc16, ident[0:BL, 0:BL])
        t16 = wk.tile([16, BL], dt)
        nc.vector.tensor_copy(out=t16, in_=ps_t16)
        cols4 = wk.tile([128, 4], dt)  # [str | m0 | m1 | m2] as rb-columns
        for q in range(4):
            nc.sync.dma_start(out=cols4[:, q:q + 1],
                              in_=t16[4 * q:4 * q + 4, :])
        str_col = cols4[:, 0:1]
        m0_col = cols4[:, 1:2]

        nc.vector.tensor_scalar(out=xrow[:, 128:256], in0=xrow[:, 128:256],
                                scalar1=wg, scalar2=None, op0=ALU.mult)
        nc.vector.tensor_scalar(out=xrow[:, 256:384], in0=wv, scalar1=wg,
                                scalar2=None, op0=ALU.mult)
        nc.vector.tensor_copy(out=xrow[:, 384:385], in_=wg)
        b_half = wk.tile([BL, 1], dt)
        nc.vector.tensor_mul(out=b_half, in0=wg, in1=agt)
        nc.vector.tensor_scalar(out=xrow[:, 385:386], in0=b_half, scalar1=0.5,
                                scalar2=None, op0=ALU.mult)

        sq = wk.tile([BL, 128], dt, tag="sq")
        nrm = wk.tile([BL, 1], dt, tag="nrm")
        nc.scalar.activation(out=sq, in_=wv, func=AF.Square, accum_out=nrm)
        nc.scalar.activation(out=nrm, in_=nrm, func=AF.Sqrt)
        nc.vector.tensor_scalar(out=nrm, in0=nrm, scalar1=1e-12, scalar2=None,
                                op0=ALU.max)
        nc.vector.reciprocal(out=nrm, in_=nrm)
        nc.vector.tensor_scalar(out=xrow[:, 0:128], in0=wv, scalar1=nrm,
                                scalar2=None, op0=ALU.mult)

        # ---- payload stores + the single AllGather ----
        nc.sync.dma_start(
            out=ag_in[P_X:P_X + BL * XW].rearrange("(p f) -> p f", p=BL),
            in_=xrow)
        nc.scalar.dma_start(
            out=ag_in[P_AL:P_AL + 256].rearrange("(t p) -> p t", p=128),
            in_=al2)
        nc.scalar.dma_start(
            out=ag_in[P_RS:P_RS + 256].rearrange("(t p) -> p t", p=128),
            in_=rs2)
        nc.sync.dma_start(out=ag_in[P_CS:P_CS + N], in_=csrow)

        nc.gpsimd.collective_compute(
            "AllGather", ALU.bypass, replica_groups=[list(range(M))],
            ins=[ag_in[:]], outs=[ag_out.flatten()])

        # ---- post-AG loads (few, fat, split across queues) ----
        xf = []
        for bc in range(2):
            t_x = wk.tile([128, XW], BF16, name=f"xf{bc}")
            for r in range(4):
                rk = bc * 4 + r
                eng = nc.sync if r % 2 == 0 else nc.scalar
                eng.dma_start(
                    out=t_x[r * BL:(r + 1) * BL, :],
                    in_=ag_out[rk, P_X:P_X + BL * XW]
                    .rearrange("(p f) -> p f", p=BL))
            xf.append(t_x)
        alrs = wk.tile([1, M * 512], BF16)
        for r in range(M):
            eng = nc.sync if r % 2 == 0 else nc.scalar
            eng.dma_start(out=alrs[0:1, r * 512:(r + 1) * 512],
                          in_=ag_out[r, P_AL:P_AL + 512])
        cs8 = wk.tile([M, N], BF16)
        nc.scalar.dma_start(out=cs8, in_=ag_out[:, P_CS:P_CS + N])

        # content-weight softmax for all 256 rows (redundant on every core)
        nwvT = []
        cwf = []
        den2 = wk.tile([128, 2], dt)
        for bc in range(2):
            ps_nt = ppt.tile([128, 128], BF16, tag="tr")
            nc.tensor.transpose(ps_nt, xf[bc][:, 0:128], ident_bf)
            t_nt = wk.tile([128, 128], BF16, name=f"nwvT{bc}")
            nc.vector.tensor_copy(out=t_nt, in_=ps_nt)
            nwvT.append(t_nt)
        memnT_bf = wk.tile([128, N], BF16, tag="big3")
        nc.vector.tensor_copy(out=memnT_bf, in_=memnT)
        for bc in range(2):
            ps_cw = ppb.tile([128, N], dt, tag="big")
            for ch in range(4):
                nc.tensor.matmul(ps_cw[:, ch * 512:(ch + 1) * 512],
                                 lhsT=nwvT[bc],
                                 rhs=memnT_bf[:, ch * 512:(ch + 1) * 512],
                                 start=True, stop=True)
            t_cw = wk.tile([128, N], BF16, name=f"cwf{bc}")
            nc.scalar.activation(out=t_cw, in_=ps_cw, func=AF.Exp,
                                 accum_out=den2[:, bc:bc + 1])
            cwf.append(t_cw)
        # a_sc = 0.5*wg/den ; rhs rows [a_sc*ev | a_sc*av]
        nc.vector.reciprocal(out=den2, in_=den2)
        asc2 = wk.tile([128, 2], dt)
        for bc in range(2):
            nc.vector.tensor_mul(out=asc2[:, bc:bc + 1],
                                 in0=xf[bc][:, 384:385],
                                 in1=den2[:, bc:bc + 1])
        nc.vector.tensor_scalar(out=asc2, in0=asc2, scalar1=0.5, scalar2=None,
                                op0=ALU.mult)
        rhs_eva = wk.tile([128, 2, 256], BF16)
        for bc in range(2):
            nc.vector.tensor_scalar(out=rhs_eva[:, bc, :],
                                    in0=xf[bc][:, 128:384],
                                    scalar1=asc2[:, bc:bc + 1],
                                    scalar2=None, op0=ALU.mult)

        # evb/avb row = sum_b b_half[b]*[ev|av][b,:]  (rank-1 outer factor)
        ps_evb = ppt.tile([1, 256], dt, tag="tr")
        for bc in range(2):
            nc.tensor.matmul(ps_evb, lhsT=xf[bc][:, 385:386],
                             rhs=xf[bc][:, 128:384], start=(bc == 0),
                             stop=(bc == 1))
        evab_r = wk.tile([1, 256], BF16)
        nc.vector.tensor_copy(out=evab_r, in_=ps_evb)

        # bw/fw rows: 0.9/N * [colsum(link), rowsum(link)]
        bwrow = wk.tile([1, N], dt)
        for ch in range(4):
            ps_cs2 = ppt.tile([1, 512], dt, tag="tr")
            nc.tensor.matmul(ps_cs2, lhsT=ones_cb[0:8, :],
                             rhs=cs8[:, ch * 512:(ch + 1) * 512],
                             start=True, stop=True)
            nc.vector.tensor_scalar(out=bwrow[:, ch * 512:(ch + 1) * 512],
                                    in0=ps_cs2, scalar1=0.9 / N,
                                    scalar2=None, op0=ALU.mult)
        fwrow = wk.tile([1, N], dt)
        for r in range(M):
            nc.vector.tensor_scalar(
                out=fwrow[:, r * 256:(r + 1) * 256],
                in0=alrs[:, r * 512 + 256:r * 512 + 512],
                scalar1=0.9 / N, scalar2=None, op0=ALU.mult)

        # ---- erase/add matmuls + mnew, pipelined per n-tile ----
        mnew = wk.tile([128, N], dt, tag="big1")
        mnew3 = mnew.rearrange("p (t d) -> p t d", d=128)
        for t in range(NT):
            ps_ea = ppt.tile([128, 256], dt, tag="tr")
            al_sl = alrs[0:1, 512 * (t // 2) + 128 * (t % 2):
                          512 * (t // 2) + 128 * (t % 2) + 128]
            for bc in range(2):
                nc.tensor.matmul(ps_ea, lhsT=cwf[bc][:, t * 128:(t + 1) * 128],
                                 rhs=rhs_eva[:, bc, :], start=(bc == 0),
                                 stop=False)
            nc.tensor.matmul(ps_ea, lhsT=al_sl, rhs=evab_r, start=False,
                             stop=True)
            f1 = wk.tile([128, 128], dt, tag="f1", bufs=2)
            nc.vector.tensor_scalar(out=f1, in0=ps_ea[:, 0:128],
                                    scalar1=-1.0 / B, scalar2=1.0,
                                    op0=ALU.mult, op1=ALU.add)
            nc.vector.tensor_mul(out=f1, in0=f1, in1=mem3[:, t, :])
            nc.vector.scalar_tensor_tensor(out=mnew3[:, t, :],
                                           in0=ps_ea[:, 128:256],
                                           scalar=1.0 / B, in1=f1,
                                           op0=ALU.mult, op1=ALU.add)

        # mnew row norms + normalized transpose
        sqf = wk.tile([128, NT, 128], dt, tag="big2")
        nc.vector.tensor_mul(out=sqf, in0=mnew3, in1=mnew3)
        nrm16 = wk.tile([128, NT], dt)
        nc.vector.tensor_reduce(out=nrm16, in_=sqf, axis=mybir.AxisListType.X,
                                op=ALU.add)
        nc.scalar.activation(out=nrm16, in_=nrm16, func=AF.Sqrt)
        nc.vector.tensor_scalar(out=nrm16, in0=nrm16, scalar1=1e-12,
                                scalar2=None, op0=ALU.max)
        nc.vector.reciprocal(out=nrm16, in_=nrm16)
        nmn = wk.tile([128, NT, 128], dt, tag="big3")
        nc.vector.tensor_tensor(
            out=nmn, in0=mnew3,
            in1=nrm16.unsqueeze(2).broadcast_to([128, NT, 128]), op=ALU.mult)
        mnewT = memnT  # reuse buffer: memnT dead after cw sim
        for t in range(NT):
            ps_t = ppt.tile([128, 128], dt, tag="tr")
            nc.tensor.transpose(ps_t, nmn[:, t, :], ident)
            nc.scalar.copy(out=mnewT[:, t * 128:(t + 1) * 128], in_=ps_t)

        # ---- read addressing (own 32 rows; rb = r*32+b on partitions) ----
        nkT = wk.tile([128, 128], dt)
        rk3 = itf[:, 275:787].rearrange("p (r d) -> p r d", d=128)
        sqk3 = wk.tile([BL, R, 128], dt)
        nc.vector.tensor_mul(out=sqk3, in0=rk3, in1=rk3)
        nrk4 = wk.tile([BL, R], dt)
        nc.vector.tensor_reduce(out=nrk4, in_=sqk3, axis=mybir.AxisListType.X,
                                op=ALU.add)
        nc.scalar.activation(out=nrk4, in_=nrk4, func=AF.Sqrt)
        nc.vector.tensor_scalar(out=nrk4, in0=nrk4, scalar1=1e-12,
                                scalar2=None, op0=ALU.max)
        nc.vector.reciprocal(out=nrk4, in_=nrk4)
        nc.vector.tensor_tensor(
            out=sqk3, in0=rk3,
            in1=nrk4.unsqueeze(2).broadcast_to([BL, R, 128]), op=ALU.mult)
        for r in range(R):
            ps_k = ppt.tile([128, BL], dt, tag="tr")
            nc.tensor.transpose(ps_k, sqk3[:, r, :], ident[0:BL, 0:BL])
            nc.vector.tensor_copy(out=nkT[:, r * BL:(r + 1) * BL], in_=ps_k)

        ps_sim = ppb.tile([128, N], dt, tag="big")
        for ch in range(4):
            nc.tensor.matmul(ps_sim[:, ch * 512:(ch + 1) * 512], lhsT=nkT,
                             rhs=mnewT[:, ch * 512:(ch + 1) * 512],
                             start=True, stop=True)
        esim = wk.tile([128, N], dt, tag="big2")
        dsum = wk.tile([128, 1], dt)
        nc.scalar.activation(out=esim, in_=ps_sim, func=AF.Exp, scale=str_col,
                             accum_out=dsum)
        nc.vector.reciprocal(out=dsum, in_=dsum)
        c0 = wk.tile([128, 1], dt)
        nc.vector.tensor_mul(out=c0, in0=m0_col, in1=dsum)
        ps_m1 = ppt.tile([1, 128], dt, tag="tr")
        nc.tensor.transpose(ps_m1, cols4[:, 2:3], ident)
        m1T = wk.tile([1, 128], dt)
        nc.vector.tensor_copy(out=m1T, in_=ps_m1)
        ps_m2 = ppt.tile([1, 128], dt, tag="tr")
        nc.tensor.transpose(ps_m2, cols4[:, 3:4], ident)
        m2T = wk.tile([1, 128], dt)
        nc.vector.tensor_copy(out=m2T, in_=ps_m2)
        ps_term = ppb.tile([128, N], dt, tag="big")
        for ch in range(4):
            nc.tensor.matmul(ps_term[:, ch * 512:(ch + 1) * 512], lhsT=m1T,
                             rhs=bwrow[:, ch * 512:(ch + 1) * 512],
                             start=True, stop=False)
            nc.tensor.matmul(ps_term[:, ch * 512:(ch + 1) * 512], lhsT=m2T,
                             rhs=fwrow[:, ch * 512:(ch + 1) * 512],
                             start=False, stop=True)
        nrw = esim
        for ch in range(4):
            nc.vector.scalar_tensor_tensor(
                out=nrw[:, ch * 512:(ch + 1) * 512],
                in0=esim[:, ch * 512:(ch + 1) * 512], scalar=c0,
                in1=ps_term[:, ch * 512:(ch + 1) * 512], op0=ALU.mult,
                op1=ALU.add)

        ps_ro = pp1.tile([128, 128], dt, tag="p1")
        roT = wk.tile([128, 128], BF16)
        for t in range(NT):
            ps_tr = ppt.tile([128, 128], dt, tag="tr")
            nc.tensor.transpose(ps_tr, nrw[:, t * 128:(t + 1) * 128], ident)
            nrwT = wk.tile([128, 128], dt, tag="nrwT", bufs=2)
            nc.scalar.copy(out=nrwT, in_=ps_tr)
            nc.tensor.matmul(ps_ro, lhsT=mnew[:, t * 128:(t + 1) * 128],
                             rhs=nrwT, start=(t == 0), stop=(t == NT - 1))
        nc.vector.tensor_copy(out=roT, in_=ps_ro)

        # ---- output projection read-part + writeback ----
        ps_f2 = pp1.tile([BL, 1024], dt, tag="p1")
        for k in range(4):
            for hf in range(2):
                nc.tensor.matmul(
                    ps_f2[:, hf * 512:(hf + 1) * 512],
                    lhsT=roT[:, k * BL:(k + 1) * BL],
                    rhs=woutsb[:, (KH + k) * 1024 + hf * 512:
                               (KH + k) * 1024 + (hf + 1) * 512],
                    start=(k == 0), stop=(k == 3))
        nc.vector.tensor_tensor(out=outacc, in0=ps_f2, in1=outacc, op=ALU.add)
        nc.vector.tensor_tensor(out=outacc, in0=outacc, in1=bout_bc,
                                op=ALU.add)
        nc.sync.dma_start(out=outF[:, :], in_=outacc)

        if DEBUG:
            nc.sync.dma_start(out=d_h[:, :], in_=hrow)
            nc.sync.dma_start(out=d_itf[:, :], in_=itf)
            d_cwf = wk.tile([128, N], dt)
            nc.vector.tensor_copy(out=d_cwf, in_=cwf[0])
            nc.sync.dma_start(out=d_cw[:, :], in_=d_cwf)
            nc.sync.dma_start(out=d_mnew[:, :], in_=mnew)
            nc.sync.dma_start(out=d_nrw[:, :], in_=nrw)
            nc.sync.dma_start(out=d_bw[0:1, :], in_=bwrow)
            nc.sync.dma_start(out=d_bw[1:2, :], in_=fwrow)

    nc.finalize()
    return nc


def _prep_inputs(x, memory, usage, link, W_ih, W_hh, b_ih, b_hh, W_if, b_if,
                 W_out, b_out):
    f = np.float32
    x = np.asarray(x, f); memory = np.asarray(memory, f)
    usage = np.asarray(usage, f); link = np.asarray(link, f)
    W_ih = np.asarray(W_ih, f); b_ih = np.asarray(b_ih, f)
    b_hh = np.asarray(b_hh, f); W_if = np.asarray(W_if, f)
    b_if = np.asarray(b_if, f); W_out = np.asarray(W_out, f)
    b_out = np.asarray(b_out, f)

    sel = np.r_[0:1024, 2048:4096]
    W3T = W_ih[sel].T                             # (1536, 3072)
    w3r = np.ascontiguousarray(
        W3T.reshape(KC, 128, 3072).transpose(1, 0, 2)
        .reshape(128, KC * 3072).astype(ml_dtypes.bfloat16))
    b3row = np.ascontiguousarray(
        (b_ih + b_hh)[sel].reshape(1, 3072).astype(ml_dtypes.bfloat16))
    wifr = np.ascontiguousarray(
        W_if.T.reshape(KH, 128, IF).transpose(1, 0, 2)
        .reshape(128, KH * IF).astype(ml_dtypes.bfloat16))
    woutr = np.ascontiguousarray(
        W_out.T.reshape(KC, 128, 1024).transpose(1, 0, 2)
        .reshape(128, KC * 1024).astype(ml_dtypes.bfloat16))
    boutr = b_out.reshape(1, 1024).astype(ml_dtypes.bfloat16)
    memA = np.ascontiguousarray(
        memory.reshape(NT, 128, 128).transpose(1, 0, 2).reshape(128, N))
    bifr = b_if.reshape(1, IF).astype(ml_dtypes.bfloat16)
    usgr = usage.reshape(1, N)

    shared = dict(w3r=w3r, b3row=b3row, wifr=wifr, bifr=bifr, woutr=woutr,
                  boutr=boutr, memA=memA, usg=usgr)
    in_maps = []
    for c in range(M):
        xs = x[c * BL:(c + 1) * BL]               # (32, 1024)
        cinx = np.ascontiguousarray(
            xs.T.reshape(KH, 128, BL).transpose(1, 0, 2)
            .reshape(128, KH * BL).astype(ml_dtypes.bfloat16))
        ls = link[c * 256:(c + 1) * 256]          # (256, 2048)
        lnkm = np.ascontiguousarray(
            ls.reshape(2, 128, N).transpose(1, 0, 2)
            .reshape(128, 2 * N).astype(ml_dtypes.bfloat16))
        ucols = np.ascontiguousarray(
            usage.reshape(NT, 128)[2 * c:2 * c + 2].T)      # (128, 2)
        m = dict(shared)
        m["cinx"] = cinx
        m["lnk"] = lnkm
        m["ucols"] = ucols
        in_maps.append(m)
    return in_maps


def kernel(**inputs):
    nc = build_nc()
    in_maps = _prep_inputs(**inputs)
    res = run_bass_kernel_spmd(nc, in_maps, list(range(M))).results
    return np.concatenate([res[c]["outF"] for c in range(M)],
                          0).astype(np.float32)


# revision 50
# speedup vs baseline: 1.2913x; 1.2669x over previous
"""DNC single-step forward on 8 Trainium2 NeuronCores (Bass/Tile) — v3.

Data-parallel over batch (B=256 -> 32/core); params replicated.
SINGLE collective: one ~30KB/rank AllGather (Mesh) of per-core write
factors [nwv | ev | av | wg,b_half | alloc shard | link rowsum shard |
link colsum partial]. Every core then redundantly recomputes the full
(B,N) content weights and the (N,D) erase/add matrices from the
gathered factors — cheaper than AllReducing the 1MB matrices, and the
only cross-core serialization left is the once-per-execution CC
barrier.

Math restructuring (validated vs reference in numpy at ~3e-3):
  - h0=c0=0  =>  W_hh and the forget gate are dead.
  - prev_rw uniform 1/N  =>  read_flat = tiled col-mean(memory); the
    backward/forward einsums collapse to (1/N)*col/row-sums of link_new.
  - the lu (ww^T ww) link update contributes ~1e-7 rel -> dropped.
  - write_w[b,:] = a_sc[b]*cwexp[b,:] + b_half[b]*alloc[:], so
    erase_mat = cwexp^T @ (a_sc*ev) + alloc outer (b_half . ev); the
    write weights are never materialized.
  - allocation weights sort-free: alloc[i] = u[i]*exp(sum_{u_k<u_i} ln(1-u_k)).
"""
import sys

sys.path.insert(0, '/opt/trn_rl_repo')

import numpy as np
import ml_dtypes
import concourse.bass as bass
import concourse.bacc as bacc
import concourse.tile as tile
from concourse import mybir
from concourse.bass_utils import run_bass_kernel_spmd
from concourse.masks import make_identity

AF = mybir.ActivationFunctionType
ALU = mybir.AluOpType
F32 = mybir.dt.float32
BF16 = mybir.dt.bfloat16

B, N, D, R, H, I = 256, 2048, 128, 4, 1024, 1024
CI = I + R * D          # 1536
IF = 787
M = 8                   # cores
BL = B // M             # 32 batch rows per core
NT = N // 128           # 16 n-tiles
KC = CI // 128          # 12 k-tiles of cin
KH = H // 128           # 8 k-tiles of h

# AllGather payload (bf16 elements, per core)
# X section, row-major [32, 388]: [nwv(128) | ev(128) | av(128) | c4(4)]
XW = 388
P_X = 0
P_AL = P_X + BL * XW           # alloc shard, (i p) flat, 256
P_RS = P_AL + 256              # link rowsum shard, (i p) flat, 256
P_CS = P_RS + 256              # link colsum partial, 2048
PAY = P_CS + N                 # 14976 el = 29952 B/rank -> Mesh

DEBUG = False


def build_nc():
    nc = bacc.Bacc("TRN2", target_bir_lowering=False, num_devices=M)
    dt = F32
    # ---- inputs (host-prepared layouts) ----
    cinx = nc.declare_dram_parameter("cinx", [128, KH * BL], BF16, isOutput=False)
    w3r = nc.declare_dram_parameter("w3r", [128, KC * 3072], BF16, isOutput=False)
    b3row = nc.declare_dram_parameter("b3row", [1, 3072], BF16, isOutput=False)
    wifr = nc.declare_dram_parameter("wifr", [128, KH * IF], BF16, isOutput=False)
    bifr = nc.declare_dram_parameter("bifr", [1, IF], BF16, isOutput=False)
    woutr = nc.declare_dram_parameter("woutr", [128, KC * 1024], BF16, isOutput=False)
    boutr = nc.declare_dram_parameter("boutr", [1, 1024], BF16, isOutput=False)
    memA = nc.declare_dram_parameter("memA", [128, N], dt, isOutput=False)
    lnk = nc.declare_dram_parameter("lnk", [128, 2 * N], BF16, isOutput=False)
    usg = nc.declare_dram_parameter("usg", [1, N], dt, isOutput=False)
    ucols = nc.declare_dram_parameter("ucols", [128, 2], dt, isOutput=False)

    outF = nc.declare_dram_parameter("outF", [BL, 1024], dt, isOutput=True)
    if DEBUG:
        d_h = nc.declare_dram_parameter("d_h", [BL, 1024], dt, isOutput=True)
        d_itf = nc.declare_dram_parameter("d_itf", [BL, IF], dt, isOutput=True)
        d_cw = nc.declare_dram_parameter("d_cw", [128, N], dt, isOutput=True)
        d_mnew = nc.declare_dram_parameter("d_mnew", [128, N], dt, isOutput=True)
        d_nrw = nc.declare_dram_parameter("d_nrw", [128, N], dt, isOutput=True)
        d_bw = nc.declare_dram_parameter("d_bw", [2, N], dt, isOutput=True)

    from contextlib import ExitStack
    with tile.TileContext(nc) as tc, ExitStack() as es:
        cons = es.enter_context(tc.tile_pool(name="cons", bufs=1))
        wk = es.enter_context(tc.tile_pool(name="wk", bufs=1))
        wstr = es.enter_context(tc.tile_pool(name="wstr", bufs=4))
        dpool = es.enter_context(tc.tile_pool(name="dram", bufs=1, space="DRAM"))

        ag_in = dpool.tile([PAY], BF16)
        ag_out = dpool.tile([M, PAY], BF16, addr_space="Shared")

        # ---- small time-critical DMAs first (don't starve behind weights) --
        cin = wk.tile([128, KH, BL], BF16)
        nc.sync.dma_start(out=cin,
                          in_=cinx[:, :].rearrange("p (k b) -> p k b", b=BL))
        b3sb = cons.tile([1, 3072], BF16)
        nc.sync.dma_start(out=b3sb, in_=b3row[:, :])
        bif_sb = cons.tile([1, IF], BF16)
        nc.sync.dma_start(out=bif_sb, in_=bifr[:, :])
        bout_sb = cons.tile([1, 1024], BF16)
        nc.sync.dma_start(out=bout_sb, in_=boutr[:, :])
        mem_sb = cons.tile([128, N], dt)
        nc.sync.dma_start(out=mem_sb, in_=memA[:, :])
        mem3 = mem_sb.rearrange("p (t d) -> p t d", d=128)
        lt2 = cons.tile([128, 2, N], BF16)
        for i in range(2):
            nc.sync.dma_start(out=lt2[:, i, :], in_=lnk[:, i * N:(i + 1) * N])
        usg_b = wk.tile([128, N], dt, tag="big4")
        nc.sync.dma_start(out=usg_b, in_=usg[0:1, :].partition_broadcast(128))
        uc_sb = cons.tile([128, 2], dt)
        nc.sync.dma_start(out=uc_sb, in_=ucols[:, :])
        # ---- bulk weights: w3 first (consumed first), then wif, then wout --
        w3k_t = []
        for k in range(KC):
            w3k = wstr.tile([128, 3072], BF16, tag="w3k")
            for q in range(3):
                eng = nc.sync if (k * 3 + q) % 2 == 0 else nc.scalar
                eng.dma_start(
                    out=w3k[:, q * 1024:(q + 1) * 1024],
                    in_=w3r[:, k * 3072 + q * 1024:k * 3072 + (q + 1) * 1024])
            w3k_t.append(w3k)
        wif_t = []
        for k in range(KH):
            wf_k = cons.tile([128, IF], BF16, name=f"wif{k}")
            eng = nc.sync if k % 2 == 0 else nc.scalar
            eng.dma_start(out=wf_k, in_=wifr[:, k * IF:(k + 1) * IF])
            wif_t.append(wf_k)
        woutsb = cons.tile([128, KC * 1024], BF16)
        for k in range(KC):
            eng = nc.sync if k % 2 == 0 else nc.scalar
            eng.dma_start(out=woutsb[:, k * 1024:(k + 1) * 1024],
                          in_=woutr[:, k * 1024:(k + 1) * 1024])

        ident = cons.tile([128, 128], dt)
        make_identity(nc, ident)
        ident_bf = cons.tile([128, 128], BF16)
        make_identity(nc, ident_bf)
        ones_col = cons.tile([128, 1], dt)
        nc.vector.memset(ones_col, 1.0)
        ones_cb = cons.tile([128, 1], BF16)
        nc.vector.memset(ones_cb, 1.0)
        ones32 = cons.tile([128, BL], dt)
        nc.vector.memset(ones32, 1.0)
        ones1b = cons.tile([1, BL], BF16)
        nc.vector.memset(ones1b, 1.0)

        hT = cons.tile([128, KH, BL], BF16)
        memnT = cons.tile([128, N], BF16)
        csrow = wk.tile([1, N], BF16)
        al2 = wk.tile([128, 2], BF16)
        rs2 = wk.tile([128, 2], BF16)

        # ======== phase 1 (psG psum scope): gates + transposes + colsums ====
        with tc.tile_pool(name="psG", bufs=1, space="PSUM") as psG:
            # col-mean of memory -> cmean stationary [128(d), 32(b)]
            ps_mc = psG.tile([128, 128], dt, tag="tr", bufs=2)
            for t in range(NT):
                nc.tensor.matmul(ps_mc[:, 0:1], lhsT=mem3[:, t, :],
                                 rhs=ones_col, start=(t == 0),
                                 stop=(t == NT - 1))
            mean_col = wk.tile([128, 1], dt)
            nc.scalar.activation(out=mean_col, in_=ps_mc[:, 0:1], func=AF.Copy,
                                 scale=1.0 / N)
            cmean = wk.tile([128, BL], BF16)
            nc.scalar.activation(out=cmean, in_=ones32, func=AF.Copy,
                                 scale=mean_col)

            # LSTM gates, transposed: out [32(b), 3072(j)] in 6 psum banks
            ps_g = psG.tile([BL, 3072], dt, tag="g")
            for k in range(KC):
                lhs_k = cin[:, k, :] if k < KH else cmean
                for g6 in range(6):
                    nc.tensor.matmul(
                        ps_g[:, g6 * 512:(g6 + 1) * 512], lhsT=lhs_k,
                        rhs=w3k_t[k][:, g6 * 512:(g6 + 1) * 512],
                        start=(k == 0), stop=False)
            for g6 in range(6):   # bias via rank-1 ones x b3row
                nc.tensor.matmul(ps_g[:, g6 * 512:(g6 + 1) * 512],
                                 lhsT=ones1b,
                                 rhs=b3sb[0:1, g6 * 512:(g6 + 1) * 512],
                                 start=False, stop=True)
            graw = wk.tile([BL, 3072], BF16)
            nc.scalar.activation(out=graw[:, 0:1024], in_=ps_g[:, 0:1024],
                                 func=AF.Sigmoid)
            nc.scalar.activation(out=graw[:, 2048:3072], in_=ps_g[:, 2048:3072],
                                 func=AF.Sigmoid)
            nc.scalar.activation(out=graw[:, 1024:2048], in_=ps_g[:, 1024:2048],
                                 func=AF.Tanh)
            hrow = wk.tile([BL, 1024], dt)
            nc.vector.tensor_mul(out=hrow, in0=graw[:, 0:1024],
                                 in1=graw[:, 1024:2048])
            nc.scalar.activation(out=hrow, in_=hrow, func=AF.Tanh)
            nc.vector.tensor_mul(out=hrow, in0=hrow, in1=graw[:, 2048:3072])
            for t in range(KH):
                ps_t = psG.tile([128, 128], dt, tag="tr", bufs=2)
                nc.tensor.transpose(ps_t[:, 0:BL],
                                    hrow[:, t * 128:(t + 1) * 128],
                                    ident[0:BL, 0:BL])
                nc.vector.tensor_copy(out=hT[:, t, :], in_=ps_t[:, 0:BL])

            # normalized memory rows, transposed -> memnT [128(d), N]
            sqm3 = wk.tile([128, NT, 128], dt, tag="big1")
            nc.vector.tensor_mul(out=sqm3, in0=mem3, in1=mem3)
            mn16 = wk.tile([128, NT], dt)
            nc.vector.tensor_reduce(out=mn16, in_=sqm3,
                                    axis=mybir.AxisListType.X, op=ALU.add)
            nc.scalar.activation(out=mn16, in_=mn16, func=AF.Sqrt)
            nc.vector.tensor_scalar(out=mn16, in0=mn16, scalar1=1e-12,
                                    scalar2=None, op0=ALU.max)
            nc.vector.reciprocal(out=mn16, in_=mn16)
            nmn1 = wk.tile([128, NT, 128], BF16, tag="bigb1")
            nc.vector.tensor_tensor(
                out=nmn1, in0=mem3,
                in1=mn16.unsqueeze(2).broadcast_to([128, NT, 128]),
                op=ALU.mult)
            for t in range(NT):
                ps_t = psG.tile([128, 128], BF16, tag="tr", bufs=2)
                nc.tensor.transpose(ps_t, nmn1[:, t, :], ident_bf)
                nc.vector.tensor_copy(out=memnT[:, t * 128:(t + 1) * 128],
                                      in_=ps_t)

            # link colsum partial (this core's 256 rows), via ones matmuls
            for ch in range(4):
                ps_cs = psG.tile([1, 512], dt, tag="tr", bufs=2)
                for i in range(2):
                    nc.tensor.matmul(ps_cs, lhsT=ones_cb,
                                     rhs=lt2[:, i, ch * 512:(ch + 1) * 512],
                                     start=(i == 0), stop=(i == 1))
                nc.scalar.copy(out=csrow[:, ch * 512:(ch + 1) * 512],
                               in_=ps_cs)

        # ---- link rowsum shard + allocation shard (DVE/scalar only) ----
        for i in range(2):
            rcol = wk.tile([128, 1], dt, tag="rcol", bufs=2)
            nc.vector.tensor_reduce(out=rcol, in_=lt2[:, i, :],
                                    axis=mybir.AxisListType.X, op=ALU.add)
            nc.vector.tensor_copy(out=rs2[:, i:i + 1], in_=rcol)
        L_b = wk.tile([128, N], dt, tag="big2")
        nc.scalar.activation(out=L_b, in_=usg_b, func=AF.Ln, bias=1.0,
                             scale=-1.0)
        for i in range(2):
            u_col = uc_sb[:, i:i + 1]
            step = wk.tile([128, N], dt, tag="big3")
            nc.vector.tensor_scalar(out=step, in0=usg_b, scalar1=u_col,
                                    scalar2=None, op0=ALU.is_lt)
            nc.vector.tensor_tensor(out=step, in0=step, in1=L_b, op=ALU.mult)
            a_col = wk.tile([128, 1], dt, tag="acol", bufs=2)
            nc.vector.tensor_reduce(out=a_col, in_=step,
                                    axis=mybir.AxisListType.X, op=ALU.add)
            nc.scalar.activation(out=a_col, in_=a_col, func=AF.Exp)
            nc.vector.tensor_mul(out=a_col, in0=a_col, in1=u_col)
            nc.vector.tensor_copy(out=al2[:, i:i + 1], in_=a_col)

        # ======== phase 2 (ppb/ppt/pp1 psum scope): rest of kernel ========
        ppb = es.enter_context(tc.tile_pool(name="ppb", bufs=1, space="PSUM"))
        ppt = es.enter_context(tc.tile_pool(name="ppt", bufs=2, space="PSUM"))
        pp1 = es.enter_context(tc.tile_pool(name="pp1", bufs=1, space="PSUM"))

        # interface vector for own 32 rows
        ps_itf = ppb.tile([BL, IF], dt, tag="big")
        for k in range(KH):
            nc.tensor.matmul(ps_itf[:, 0:512], lhsT=hT[:, k, :],
                             rhs=wif_t[k][:, 0:512], start=(k == 0),
                             stop=False)
            nc.tensor.matmul(ps_itf[:, 512:IF], lhsT=hT[:, k, :],
                             rhs=wif_t[k][:, 512:IF], start=(k == 0),
                             stop=False)
        nc.tensor.matmul(ps_itf[:, 0:512], lhsT=ones1b,
                         rhs=bif_sb[0:1, 0:512], start=False, stop=True)
        nc.tensor.matmul(ps_itf[:, 512:IF], lhsT=ones1b,
                         rhs=bif_sb[0:1, 512:IF], start=False, stop=True)
        itf = wk.tile([BL, IF], dt)
        nc.vector.tensor_copy(out=itf, in_=ps_itf)

        # output-projection h-part, accumulated now (off critical path)
        ps_fh = pp1.tile([BL, 1024], dt, tag="p1")
        for k in range(KH):
            for hf in range(2):
                nc.tensor.matmul(
                    ps_fh[:, hf * 512:(hf + 1) * 512], lhsT=hT[:, k, :],
                    rhs=woutsb[:, k * 1024 + hf * 512:k * 1024 + (hf + 1) * 512],
                    start=(k == 0), stop=False)
        for hf in range(2):   # + b_out via rank-1 ones x bias row
            nc.tensor.matmul(ps_fh[:, hf * 512:(hf + 1) * 512], lhsT=ones1b,
                             rhs=bout_sb[0:1, hf * 512:(hf + 1) * 512],
                             start=False, stop=True)
        outacc = cons.tile([BL, 1024], dt)
        nc.vector.tensor_copy(out=outacc, in_=ps_fh)

        # ---- write addressing factors -> X payload row [32, 388] ----
        xrow = wk.tile([BL, XW], BF16)
        wv = itf[:, 0:128]
        nc.scalar.activation(out=xrow[:, 128:256], in_=itf[:, 128:256],
                             func=AF.Sigmoid)    # sig(erase), *wg below
        wgag = wk.tile([BL, 2], dt)
        nc.scalar.activation(out=wgag, in_=itf[:, 256:258], func=AF.Sigmoid)
        wg = wgag[:, 0:1]
        agt = wgag[:, 1:2]
        expm = wk.tile([BL, 12], dt)
        nc.scalar.activation(out=expm, in_=itf[:, 259:271], func=AF.Exp)
        msum = wk.tile([BL, 4], dt)
        nc.vector.tensor_reduce(out=msum,
                                in_=expm.rearrange("p (r k) -> p r k", k=3),
                                axis=mybir.AxisListType.X, op=ALU.add)
        minv = wk.tile([BL, 4], dt)
        nc.vector.reciprocal(out=minv, in_=msum)
        sc16 = wk.tile([BL, 16], dt)   # [rstr | m0 | m1 | m2]
        nc.scalar.activation(out=sc16[:, 0:4], in_=itf[:, 271:275],
                             func=AF.Exp)
        nc.scalar.activation(out=sc16[:, 0:4], in_=sc16[:, 0:4],
                             func=AF.Ln, bias=1.0)
        em3 = expm.rearrange("p (r k) -> p r k", k=3)
        for kk in range(3):
            nc.vector.tensor_mul(out=sc16[:, 4 + 4 * kk:8 + 4 * kk],
                                 in0=em3[:, :, kk], in1=minv)
        ps_t16 = ppt.tile([16, BL], dt, tag="tr")
        nc.tensor.transpose(ps_t16, sc16, ident[0:BL, 0:BL])
        t16 = wk.tile([16, BL], dt)
        nc.vector.tensor_copy(out=t16, in_=ps_t16)
        cols4 = wk.tile([128, 4], dt)  # [str | m0 | m1 | m2] as rb-columns
        for q in range(4):
            nc.sync.dma_start(out=cols4[:, q:q + 1],
                              in_=t16[4 * q:4 * q + 4, :])
        str_col = cols4[:, 0:1]
        m0_col = cols4[:, 1:2]

        nc.vector.tensor_scalar(out=xrow[:, 128:256], in0=xrow[:, 128:256],
                                scalar1=wg, scalar2=None, op0=ALU.mult)
        nc.vector.tensor_scalar(out=xrow[:, 256:384], in0=wv, scalar1=wg,
                                scalar2=None, op0=ALU.mult)
        nc.vector.tensor_copy(out=xrow[:, 384:385], in_=wg)
        b_half = wk.tile([BL, 1], dt)
        nc.vector.tensor_mul(out=b_half, in0=wg, in1=agt)
        nc.vector.tensor_scalar(out=xrow[:, 385:386], in0=b_half, scalar1=0.5,
                                scalar2=None, op0=ALU.mult)

        sq = wk.tile([BL, 128], dt, tag="sq")
        nrm = wk.tile([BL, 1], dt, tag="nrm")
        nc.scalar.activation(out=sq, in_=wv, func=AF.Square, accum_out=nrm)
        nc.scalar.activation(out=nrm, in_=nrm, func=AF.Sqrt)
        nc.vector.tensor_scalar(out=nrm, in0=nrm, scalar1=1e-12, scalar2=None,
                                op0=ALU.max)
        nc.vector.reciprocal(out=nrm, in_=nrm)
        nc.vector.tensor_scalar(out=xrow[:, 0:128], in0=wv, scalar1=nrm,
                                scalar2=None, op0=ALU.mult)

        # read-key norms + mode-column transposes: no AG dependency, do now
        nkT = wk.tile([128, 128], BF16)
        rk3 = itf[:, 275:787].rearrange("p (r d) -> p r d", d=128)
        sqk3 = wk.tile([BL, R, 128], dt)
        nc.vector.tensor_mul(out=sqk3, in0=rk3, in1=rk3)
        nrk4 = wk.tile([BL, R], dt)
        nc.vector.tensor_reduce(out=nrk4, in_=sqk3, axis=mybir.AxisListType.X,
                                op=ALU.add)
        nc.scalar.activation(out=nrk4, in_=nrk4, func=AF.Sqrt)
        nc.vector.tensor_scalar(out=nrk4, in0=nrk4, scalar1=1e-12,
                                scalar2=None, op0=ALU.max)
        nc.vector.reciprocal(out=nrk4, in_=nrk4)
        sqk3b = wk.tile([BL, R, 128], BF16)
        nc.vector.tensor_tensor(
            out=sqk3b, in0=rk3,
            in1=nrk4.unsqueeze(2).broadcast_to([BL, R, 128]), op=ALU.mult)
        for r in range(R):
            ps_k = ppt.tile([128, BL], BF16, tag="tr")
            nc.tensor.transpose(ps_k, sqk3b[:, r, :], ident_bf[0:BL, 0:BL])
            nc.vector.tensor_copy(out=nkT[:, r * BL:(r + 1) * BL], in_=ps_k)
        ps_m1 = ppt.tile([1, 128], dt, tag="tr")
        nc.tensor.transpose(ps_m1, cols4[:, 2:3], ident)
        m1T = wk.tile([1, 128], BF16)
        nc.vector.tensor_copy(out=m1T, in_=ps_m1)
        ps_m2 = ppt.tile([1, 128], dt, tag="tr")
        nc.tensor.transpose(ps_m2, cols4[:, 3:4], ident)
        m2T = wk.tile([1, 128], BF16)
        nc.vector.tensor_copy(out=m2T, in_=ps_m2)

        # ---- payload stores + the single AllGather ----
        nc.sync.dma_start(
            out=ag_in[P_X:P_X + BL * XW].rearrange("(p f) -> p f", p=BL),
            in_=xrow)
        nc.scalar.dma_start(
            out=ag_in[P_AL:P_AL + 256].rearrange("(t p) -> p t", p=128),
            in_=al2)
        nc.scalar.dma_start(
            out=ag_in[P_RS:P_RS + 256].rearrange("(t p) -> p t", p=128),
            in_=rs2)
        nc.sync.dma_start(out=ag_in[P_CS:P_CS + N], in_=csrow)

        nc.gpsimd.collective_compute(
            "AllGather", ALU.bypass, replica_groups=[list(range(M))],
            ins=[ag_in[:]], outs=[ag_out.flatten()])

        # ---- post-AG loads (few, fat, split across queues) ----
        xf = []
        for bc in range(2):
            t_x = wk.tile([128, XW], BF16, name=f"xf{bc}")
            for r in range(4):
                rk = bc * 4 + r
                eng = nc.sync if r % 2 == 0 else nc.scalar
                eng.dma_start(
                    out=t_x[r * BL:(r + 1) * BL, :],
                    in_=ag_out[rk, P_X:P_X + BL * XW]
                    .rearrange("(p f) -> p f", p=BL))
            xf.append(t_x)
        alrs = wk.tile([1, M * 512], BF16)
        for r in range(M):
            eng = nc.sync if r % 2 == 0 else nc.scalar
            eng.dma_start(out=alrs[0:1, r * 512:(r + 1) * 512],
                          in_=ag_out[r, P_AL:P_AL + 512])
        cs8 = wk.tile([M, N], BF16)
        nc.scalar.dma_start(out=cs8, in_=ag_out[:, P_CS:P_CS + N])

        # content-weight softmax for all 256 rows (redundant on every core)
        nwvT = []
        cwf = []
        den2 = wk.tile([128, 2], dt)
        for bc in range(2):
            ps_nt = ppt.tile([128, 128], BF16, tag="tr")
            nc.tensor.transpose(ps_nt, xf[bc][:, 0:128], ident_bf)
            t_nt = wk.tile([128, 128], BF16, name=f"nwvT{bc}")
            nc.vector.tensor_copy(out=t_nt, in_=ps_nt)
            nwvT.append(t_nt)
        for bc in range(2):
            ps_cw = ppb.tile([128, N], dt, tag="big")
            for ch in range(4):
                nc.tensor.matmul(ps_cw[:, ch * 512:(ch + 1) * 512],
                                 lhsT=nwvT[bc],
                                 rhs=memnT[:, ch * 512:(ch + 1) * 512],
                                 start=True, stop=True)
            t_cw = wk.tile([128, N], BF16, name=f"cwf{bc}")
            nc.scalar.activation(out=t_cw, in_=ps_cw, func=AF.Exp,
                                 accum_out=den2[:, bc:bc + 1])
            cwf.append(t_cw)
        # a_sc = 0.5*wg/den ; rhs rows [a_sc*ev | a_sc*av]
        nc.vector.reciprocal(out=den2, in_=den2)
        asc2 = wk.tile([128, 2], dt)
        for bc in range(2):
            nc.vector.tensor_mul(out=asc2[:, bc:bc + 1],
                                 in0=xf[bc][:, 384:385],
                                 in1=den2[:, bc:bc + 1])
        nc.vector.tensor_scalar(out=asc2, in0=asc2, scalar1=0.5, scalar2=None,
                                op0=ALU.mult)
        rhs_eva = wk.tile([128, 2, 256], BF16)
        for bc in range(2):
            nc.vector.tensor_scalar(out=rhs_eva[:, bc, :],
                                    in0=xf[bc][:, 128:384],
                                    scalar1=asc2[:, bc:bc + 1],
                                    scalar2=None, op0=ALU.mult)

        # evb/avb row = sum_b b_half[b]*[ev|av][b,:]  (rank-1 outer factor)
        ps_evb = ppt.tile([1, 256], dt, tag="tr")
        for bc in range(2):
            nc.tensor.matmul(ps_evb, lhsT=xf[bc][:, 385:386],
                             rhs=xf[bc][:, 128:384], start=(bc == 0),
                             stop=(bc == 1))
        evab_r = wk.tile([1, 256], BF16)
        nc.vector.tensor_copy(out=evab_r, in_=ps_evb)

        # bw/fw rows: 0.9/N * [colsum(link), rowsum(link)]
        bwrow = wk.tile([1, N], BF16)
        for ch in range(4):
            ps_cs2 = ppt.tile([1, 512], dt, tag="tr")
            nc.tensor.matmul(ps_cs2, lhsT=ones_cb[0:8, :],
                             rhs=cs8[:, ch * 512:(ch + 1) * 512],
                             start=True, stop=True)
            nc.vector.tensor_scalar(out=bwrow[:, ch * 512:(ch + 1) * 512],
                                    in0=ps_cs2, scalar1=0.9 / N,
                                    scalar2=None, op0=ALU.mult)
        fwrow = wk.tile([1, N], BF16)
        for r in range(M):
            nc.vector.tensor_scalar(
                out=fwrow[:, r * 256:(r + 1) * 256],
                in0=alrs[:, r * 512 + 256:r * 512 + 512],
                scalar1=0.9 / N, scalar2=None, op0=ALU.mult)

        # ---- erase/add matmuls + mnew, pipelined per n-tile ----
        mnew = wk.tile([128, N], dt, tag="big1")
        mnew3 = mnew.rearrange("p (t d) -> p t d", d=128)
        for t in range(NT):
            ps_ea = ppt.tile([128, 256], dt, tag="tr")
            al_sl = alrs[0:1, 512 * (t // 2) + 128 * (t % 2):
                          512 * (t // 2) + 128 * (t % 2) + 128]
            for bc in range(2):
                nc.tensor.matmul(ps_ea, lhsT=cwf[bc][:, t * 128:(t + 1) * 128],
                                 rhs=rhs_eva[:, bc, :], start=(bc == 0),
                                 stop=False)
            nc.tensor.matmul(ps_ea, lhsT=al_sl, rhs=evab_r, start=False,
                             stop=True)
            f1 = wk.tile([128, 128], dt, tag="f1", bufs=2)
            nc.vector.tensor_scalar(out=f1, in0=ps_ea[:, 0:128],
                                    scalar1=-1.0 / B, scalar2=1.0,
                                    op0=ALU.mult, op1=ALU.add)
            nc.vector.tensor_mul(out=f1, in0=f1, in1=mem3[:, t, :])
            nc.vector.scalar_tensor_tensor(out=mnew3[:, t, :],
                                           in0=ps_ea[:, 128:256],
                                           scalar=1.0 / B, in1=f1,
                                           op0=ALU.mult, op1=ALU.add)

        # mnew row norms + normalized transpose
        sqf = wk.tile([128, NT, 128], dt, tag="big2")
        nc.vector.tensor_mul(out=sqf, in0=mnew3, in1=mnew3)
        nrm16 = wk.tile([128, NT], dt)
        nc.vector.tensor_reduce(out=nrm16, in_=sqf, axis=mybir.AxisListType.X,
                                op=ALU.add)
        nc.scalar.activation(out=nrm16, in_=nrm16, func=AF.Sqrt)
        nc.vector.tensor_scalar(out=nrm16, in0=nrm16, scalar1=1e-12,
                                scalar2=None, op0=ALU.max)
        nc.vector.reciprocal(out=nrm16, in_=nrm16)
        nmn = wk.tile([128, NT, 128], BF16, tag="bigb1")
        nc.vector.tensor_tensor(
            out=nmn, in0=mnew3,
            in1=nrm16.unsqueeze(2).broadcast_to([128, NT, 128]), op=ALU.mult)
        mnewT = memnT  # reuse buffer: memnT dead after cw sim
        for t in range(NT):
            ps_t = ppt.tile([128, 128], BF16, tag="tr")
            nc.tensor.transpose(ps_t, nmn[:, t, :], ident_bf)
            nc.scalar.copy(out=mnewT[:, t * 128:(t + 1) * 128], in_=ps_t)
        mnew_bf = wk.tile([128, N], BF16, tag="bigb2")
        nc.vector.tensor_copy(out=mnew_bf, in_=mnew)

        ps_sim = ppb.tile([128, N], dt, tag="big")
        for ch in range(4):
            nc.tensor.matmul(ps_sim[:, ch * 512:(ch + 1) * 512], lhsT=nkT,
                             rhs=mnewT[:, ch * 512:(ch + 1) * 512],
                             start=True, stop=True)
        esim = wk.tile([128, N], dt, tag="big2")
        dsum = wk.tile([128, 1], dt)
        nc.scalar.activation(out=esim, in_=ps_sim, func=AF.Exp, scale=str_col,
                             accum_out=dsum)
        nc.vector.reciprocal(out=dsum, in_=dsum)
        c0 = wk.tile([128, 1], dt)
        nc.vector.tensor_mul(out=c0, in0=m0_col, in1=dsum)
        ps_term = ppb.tile([128, N], dt, tag="big")
        for ch in range(4):
            nc.tensor.matmul(ps_term[:, ch * 512:(ch + 1) * 512], lhsT=m1T,
                             rhs=bwrow[:, ch * 512:(ch + 1) * 512],
                             start=True, stop=False)
            nc.tensor.matmul(ps_term[:, ch * 512:(ch + 1) * 512], lhsT=m2T,
                             rhs=fwrow[:, ch * 512:(ch + 1) * 512],
                             start=False, stop=True)
        nrw = esim
        for ch in range(4):
            nc.vector.scalar_tensor_tensor(
                out=nrw[:, ch * 512:(ch + 1) * 512],
                in0=esim[:, ch * 512:(ch + 1) * 512], scalar=c0,
                in1=ps_term[:, ch * 512:(ch + 1) * 512], op0=ALU.mult,
                op1=ALU.add)

        ps_ro = pp1.tile([128, 128], dt, tag="p1")
        roT = wk.tile([128, 128], BF16)
        for t in range(NT):
            ps_tr = ppt.tile([128, 128], dt, tag="tr")
            nc.tensor.transpose(ps_tr, nrw[:, t * 128:(t + 1) * 128], ident)
            nrwT = wk.tile([128, 128], BF16, tag="nrwT", bufs=2)
            nc.scalar.copy(out=nrwT, in_=ps_tr)
            nc.tensor.matmul(ps_ro, lhsT=mnew_bf[:, t * 128:(t + 1) * 128],
                             rhs=nrwT, start=(t == 0), stop=(t == NT - 1))
        nc.vector.tensor_copy(out=roT, in_=ps_ro)

        # ---- output projection read-part + writeback ----
        ps_f2 = pp1.tile([BL, 1024], dt, tag="p1")
        for k in range(4):
            for hf in range(2):
                nc.tensor.matmul(
                    ps_f2[:, hf * 512:(hf + 1) * 512],
                    lhsT=roT[:, k * BL:(k + 1) * BL],
                    rhs=woutsb[:, (KH + k) * 1024 + hf * 512:
                               (KH + k) * 1024 + (hf + 1) * 512],
                    start=(k == 0), stop=(k == 3))
        nc.vector.tensor_tensor(out=outacc, in0=ps_f2, in1=outacc, op=ALU.add)
        nc.sync.dma_start(out=outF[:, :], in_=outacc)

        if DEBUG:
            nc.sync.dma_start(out=d_h[:, :], in_=hrow)
            nc.sync.dma_start(out=d_itf[:, :], in_=itf)
            d_cwf = wk.tile([128, N], dt)
            nc.vector.tensor_copy(out=d_cwf, in_=cwf[0])
            nc.sync.dma_start(out=d_cw[:, :], in_=d_cwf)
            nc.sync.dma_start(out=d_mnew[:, :], in_=mnew)
            nc.sync.dma_start(out=d_nrw[:, :], in_=nrw)
            nc.sync.dma_start(out=d_bw[0:1, :], in_=bwrow)
            nc.sync.dma_start(out=d_bw[1:2, :], in_=fwrow)

    nc.finalize()
    return nc


def _prep_inputs(x, memory, usage, link, W_ih, W_hh, b_ih, b_hh, W_if, b_if,
                 W_out, b_out):
    f = np.float32
    x = np.asarray(x, f); memory = np.asarray(memory, f)
    usage = np.asarray(usage, f); link = np.asarray(link, f)
    W_ih = np.asarray(W_ih, f); b_ih = np.asarray(b_ih, f)
    b_hh = np.asarray(b_hh, f); W_if = np.asarray(W_if, f)
    b_if = np.asarray(b_if, f); W_out = np.asarray(W_out, f)
    b_out = np.asarray(b_out, f)

    sel = np.r_[0:1024, 2048:4096]
    W3T = W_ih[sel].T                             # (1536, 3072)
    w3r = np.ascontiguousarray(
        W3T.reshape(KC, 128, 3072).transpose(1, 0, 2)
        .reshape(128, KC * 3072).astype(ml_dtypes.bfloat16))
    b3row = np.ascontiguousarray(
        (b_ih + b_hh)[sel].reshape(1, 3072).astype(ml_dtypes.bfloat16))
    wifr = np.ascontiguousarray(
        W_if.T.reshape(KH, 128, IF).transpose(1, 0, 2)
        .reshape(128, KH * IF).astype(ml_dtypes.bfloat16))
    woutr = np.ascontiguousarray(
        W_out.T.reshape(KC, 128, 1024).transpose(1, 0, 2)
        .reshape(128, KC * 1024).astype(ml_dtypes.bfloat16))
    boutr = b_out.reshape(1, 1024).astype(ml_dtypes.bfloat16)
    memA = np.ascontiguousarray(
        memory.reshape(NT, 128, 128).transpose(1, 0, 2).reshape(128, N))
    bifr = b_if.reshape(1, IF).astype(ml_dtypes.bfloat16)
    usgr = usage.reshape(1, N)

    shared = dict(w3r=w3r, b3row=b3row, wifr=wifr, bifr=bifr, woutr=woutr,
                  boutr=boutr, memA=memA, usg=usgr)
    in_maps = []
    for c in range(M):
        xs = x[c * BL:(c + 1) * BL]               # (32, 1024)
        cinx = np.ascontiguousarray(
            xs.T.reshape(KH, 128, BL).transpose(1, 0, 2)
            .reshape(128, KH * BL).astype(ml_dtypes.bfloat16))
        ls = link[c * 256:(c + 1) * 256]          # (256, 2048)
        lnkm = np.ascontiguousarray(
            ls.reshape(2, 128, N).transpose(1, 0, 2)
            .reshape(128, 2 * N).astype(ml_dtypes.bfloat16))
        ucols = np.ascontiguousarray(
            usage.reshape(NT, 128)[2 * c:2 * c + 2].T)      # (128, 2)
        m = dict(shared)
        m["cinx"] = cinx
        m["lnk"] = lnkm
        m["ucols"] = ucols
        in_maps.append(m)
    return in_maps


def kernel(**inputs):
    nc = build_nc()
    in_maps = _prep_inputs(**inputs)
    res = run_bass_kernel_spmd(nc, in_maps, list(range(M))).results
    return np.concatenate([res[c]["outF"] for c in range(M)],
                          0).astype(np.float32)


# revision 52
# speedup vs baseline: 1.3440x; 1.0409x over previous
"""DNC single-step forward on 8 Trainium2 NeuronCores (Bass/Tile) — v3.

Data-parallel over batch (B=256 -> 32/core); params replicated.
SINGLE collective: one ~30KB/rank AllGather (Mesh) of per-core write
factors [nwv | ev | av | wg,b_half | alloc shard | link rowsum shard |
link colsum partial]. Every core then redundantly recomputes the full
(B,N) content weights and the (N,D) erase/add matrices from the
gathered factors — cheaper than AllReducing the 1MB matrices, and the
only cross-core serialization left is the once-per-execution CC
barrier.

Math restructuring (validated vs reference in numpy at ~3e-3):
  - h0=c0=0  =>  W_hh and the forget gate are dead.
  - prev_rw uniform 1/N  =>  read_flat = tiled col-mean(memory); the
    backward/forward einsums collapse to (1/N)*col/row-sums of link_new.
  - the lu (ww^T ww) link update contributes ~1e-7 rel -> dropped.
  - write_w[b,:] = a_sc[b]*cwexp[b,:] + b_half[b]*alloc[:], so
    erase_mat = cwexp^T @ (a_sc*ev) + alloc outer (b_half . ev); the
    write weights are never materialized.
  - allocation weights sort-free: alloc[i] = u[i]*exp(sum_{u_k<u_i} ln(1-u_k)).
"""
import sys

sys.path.insert(0, '/opt/trn_rl_repo')

import numpy as np
import ml_dtypes
import concourse.bass as bass
import concourse.bacc as bacc
import concourse.tile as tile
from concourse import mybir
from concourse.bass_utils import run_bass_kernel_spmd
from concourse.masks import make_identity

AF = mybir.ActivationFunctionType
ALU = mybir.AluOpType
F32 = mybir.dt.float32
BF16 = mybir.dt.bfloat16

B, N, D, R, H, I = 256, 2048, 128, 4, 1024, 1024
CI = I + R * D          # 1536
IF = 787
M = 8                   # cores
BL = B // M             # 32 batch rows per core
NT = N // 128           # 16 n-tiles
KC = CI // 128          # 12 k-tiles of cin
KH = H // 128           # 8 k-tiles of h

# AllGather payload (bf16 elements, per core)
# X section, row-major [32, 388]: [nwv(128) | ev(128) | av(128) | c4(4)]
XW = 388
P_X = 0
P_AL = P_X + BL * XW           # alloc shard, (i p) flat, 256
P_RS = P_AL + 256              # link rowsum shard, (i p) flat, 256
P_CS = P_RS + 256              # link colsum partial, 2048
PAY = P_CS + N                 # 14976 el = 29952 B/rank -> Mesh

DEBUG = False


def build_nc():
    nc = bacc.Bacc("TRN2", target_bir_lowering=False, num_devices=M)
    dt = F32
    # ---- inputs (host-prepared layouts) ----
    cinx = nc.declare_dram_parameter("cinx", [128, KH * BL], BF16, isOutput=False)
    w3r = nc.declare_dram_parameter("w3r", [128, KC * 3072], BF16, isOutput=False)
    b3row = nc.declare_dram_parameter("b3row", [1, 3072], BF16, isOutput=False)
    wifr = nc.declare_dram_parameter("wifr", [128, KH * IF], BF16, isOutput=False)
    bifr = nc.declare_dram_parameter("bifr", [1, IF], BF16, isOutput=False)
    woutr = nc.declare_dram_parameter("woutr", [128, KC * 1024], BF16, isOutput=False)
    boutr = nc.declare_dram_parameter("boutr", [1, 1024], BF16, isOutput=False)
    memA = nc.declare_dram_parameter("memA", [128, N], dt, isOutput=False)
    lnk = nc.declare_dram_parameter("lnk", [128, 2 * N], BF16, isOutput=False)
    usg = nc.declare_dram_parameter("usg", [1, N], dt, isOutput=False)
    ucols = nc.declare_dram_parameter("ucols", [128, 2], dt, isOutput=False)

    outF = nc.declare_dram_parameter("outF", [BL, 1024], dt, isOutput=True)
    if DEBUG:
        d_h = nc.declare_dram_parameter("d_h", [BL, 1024], dt, isOutput=True)
        d_itf = nc.declare_dram_parameter("d_itf", [BL, IF], dt, isOutput=True)
        d_cw = nc.declare_dram_parameter("d_cw", [128, N], dt, isOutput=True)
        d_mnew = nc.declare_dram_parameter("d_mnew", [128, N], dt, isOutput=True)
        d_nrw = nc.declare_dram_parameter("d_nrw", [128, N], dt, isOutput=True)
        d_bw = nc.declare_dram_parameter("d_bw", [2, N], dt, isOutput=True)

    from contextlib import ExitStack
    with tile.TileContext(nc) as tc, ExitStack() as es:
        cons = es.enter_context(tc.tile_pool(name="cons", bufs=1))
        wk = es.enter_context(tc.tile_pool(name="wk", bufs=1))
        wstr = es.enter_context(tc.tile_pool(name="wstr", bufs=4))
        dpool = es.enter_context(tc.tile_pool(name="dram", bufs=1, space="DRAM"))

        ag_in = dpool.tile([PAY], BF16)
        ag_out = dpool.tile([M, PAY], BF16, addr_space="Shared")

        # ---- small time-critical DMAs first (don't starve behind weights) --
        cin = wk.tile([128, KH, BL], BF16)
        nc.sync.dma_start(out=cin,
                          in_=cinx[:, :].rearrange("p (k b) -> p k b", b=BL))
        b3sb = cons.tile([1, 3072], BF16)
        nc.sync.dma_start(out=b3sb, in_=b3row[:, :])
        bif_sb = cons.tile([1, IF], BF16)
        nc.sync.dma_start(out=bif_sb, in_=bifr[:, :])
        bout_sb = cons.tile([1, 1024], BF16)
        nc.sync.dma_start(out=bout_sb, in_=boutr[:, :])
        mem_sb = cons.tile([128, N], dt)
        for q in range(4):
            nc.sync.dma_start(out=mem_sb[:, q * 512:(q + 1) * 512],
                              in_=memA[:, q * 512:(q + 1) * 512])
        mem3 = mem_sb.rearrange("p (t d) -> p t d", d=128)
        lt2 = cons.tile([128, 2, N], BF16)
        for i in range(2):
            for q in range(2):
                nc.sync.dma_start(
                    out=lt2[:, i, q * 1024:(q + 1) * 1024],
                    in_=lnk[:, i * N + q * 1024:i * N + (q + 1) * 1024])
        usg_b = wk.tile([128, N], dt, tag="big4")
        for q in range(2):
            nc.sync.dma_start(
                out=usg_b[:, q * 1024:(q + 1) * 1024],
                in_=usg[0:1, q * 1024:(q + 1) * 1024].partition_broadcast(128))
        uc_sb = cons.tile([128, 2], dt)
        nc.sync.dma_start(out=uc_sb, in_=ucols[:, :])
        # ---- bulk weights: w3 first (consumed first), then wif, then wout --
        w3k_t = []
        for k in range(KC):
            w3k = wstr.tile([128, 3072], BF16, tag="w3k")
            for q in range(3):
                eng = nc.sync if (k * 3 + q) % 2 == 0 else nc.scalar
                eng.dma_start(
                    out=w3k[:, q * 1024:(q + 1) * 1024],
                    in_=w3r[:, k * 3072 + q * 1024:k * 3072 + (q + 1) * 1024])
            w3k_t.append(w3k)
        wif_t = []
        for k in range(KH):
            wf_k = cons.tile([128, IF], BF16, name=f"wif{k}")
            eng = nc.sync if k % 2 == 0 else nc.scalar
            eng.dma_start(out=wf_k, in_=wifr[:, k * IF:(k + 1) * IF])
            wif_t.append(wf_k)
        woutsb = cons.tile([128, KC * 1024], BF16)
        for k in range(KC):
            eng = nc.sync if k % 2 == 0 else nc.scalar
            eng.dma_start(out=woutsb[:, k * 1024:(k + 1) * 1024],
                          in_=woutr[:, k * 1024:(k + 1) * 1024])

        ident = cons.tile([128, 128], dt)
        make_identity(nc, ident)
        ident_bf = cons.tile([128, 128], BF16)
        make_identity(nc, ident_bf)
        ones_col = cons.tile([128, 1], dt)
        nc.vector.memset(ones_col, 1.0)
        ones_cb = cons.tile([128, 1], BF16)
        nc.vector.memset(ones_cb, 1.0)
        ones32 = cons.tile([128, BL], dt)
        nc.vector.memset(ones32, 1.0)
        ones1b = cons.tile([1, BL], BF16)
        nc.vector.memset(ones1b, 1.0)

        hT = cons.tile([128, KH, BL], BF16)
        memnT = cons.tile([128, N], BF16)
        csrow = wk.tile([1, N], BF16)
        al2 = wk.tile([128, 2], BF16)
        rs2 = wk.tile([128, 2], BF16)

        # ======== phase 1 (psG psum scope): gates + transposes + colsums ====
        with tc.tile_pool(name="psG", bufs=1, space="PSUM") as psG:
            # col-mean of memory -> cmean stationary [128(d), 32(b)]
            ps_mc = psG.tile([128, 128], dt, tag="tr", bufs=2)
            for t in range(NT):
                nc.tensor.matmul(ps_mc[:, 0:1], lhsT=mem3[:, t, :],
                                 rhs=ones_col, start=(t == 0),
                                 stop=(t == NT - 1))
            mean_col = wk.tile([128, 1], dt)
            nc.scalar.activation(out=mean_col, in_=ps_mc[:, 0:1], func=AF.Copy,
                                 scale=1.0 / N)
            cmean = wk.tile([128, BL], BF16)
            nc.scalar.activation(out=cmean, in_=ones32, func=AF.Copy,
                                 scale=mean_col)

            # LSTM gates, transposed: out [32(b), 3072(j)] in 6 psum banks
            ps_g = psG.tile([BL, 3072], dt, tag="g")
            for k in range(KC):
                lhs_k = cin[:, k, :] if k < KH else cmean
                for g6 in range(6):
                    nc.tensor.matmul(
                        ps_g[:, g6 * 512:(g6 + 1) * 512], lhsT=lhs_k,
                        rhs=w3k_t[k][:, g6 * 512:(g6 + 1) * 512],
                        start=(k == 0), stop=False)
            for g6 in range(6):   # bias via rank-1 ones x b3row
                nc.tensor.matmul(ps_g[:, g6 * 512:(g6 + 1) * 512],
                                 lhsT=ones1b,
                                 rhs=b3sb[0:1, g6 * 512:(g6 + 1) * 512],
                                 start=False, stop=True)
            graw = wk.tile([BL, 3072], BF16)
            nc.scalar.activation(out=graw[:, 0:1024], in_=ps_g[:, 0:1024],
                                 func=AF.Sigmoid)
            nc.scalar.activation(out=graw[:, 2048:3072], in_=ps_g[:, 2048:3072],
                                 func=AF.Sigmoid)
            nc.scalar.activation(out=graw[:, 1024:2048], in_=ps_g[:, 1024:2048],
                                 func=AF.Tanh)
            hrow = wk.tile([BL, 1024], dt)
            nc.vector.tensor_mul(out=hrow, in0=graw[:, 0:1024],
                                 in1=graw[:, 1024:2048])
            nc.scalar.activation(out=hrow, in_=hrow, func=AF.Tanh)
            nc.vector.tensor_mul(out=hrow, in0=hrow, in1=graw[:, 2048:3072])
            for t in range(KH):
                ps_t = psG.tile([128, 128], dt, tag="tr", bufs=2)
                nc.tensor.transpose(ps_t[:, 0:BL],
                                    hrow[:, t * 128:(t + 1) * 128],
                                    ident[0:BL, 0:BL])
                nc.vector.tensor_copy(out=hT[:, t, :], in_=ps_t[:, 0:BL])

            # normalized memory rows, transposed -> memnT [128(d), N]
            sqm3 = wk.tile([128, NT, 128], dt, tag="big1")
            nc.vector.tensor_mul(out=sqm3, in0=mem3, in1=mem3)
            mn16 = wk.tile([128, NT], dt)
            nc.vector.tensor_reduce(out=mn16, in_=sqm3,
                                    axis=mybir.AxisListType.X, op=ALU.add)
            nc.scalar.activation(out=mn16, in_=mn16, func=AF.Sqrt)
            nc.vector.tensor_scalar(out=mn16, in0=mn16, scalar1=1e-12,
                                    scalar2=None, op0=ALU.max)
            nc.vector.reciprocal(out=mn16, in_=mn16)
            nmn1 = wk.tile([128, NT, 128], BF16, tag="bigb1")
            nc.vector.tensor_tensor(
                out=nmn1, in0=mem3,
                in1=mn16.unsqueeze(2).broadcast_to([128, NT, 128]),
                op=ALU.mult)
            for t in range(NT):
                ps_t = psG.tile([128, 128], BF16, tag="tr", bufs=2)
                nc.tensor.transpose(ps_t, nmn1[:, t, :], ident_bf)
                nc.vector.tensor_copy(out=memnT[:, t * 128:(t + 1) * 128],
                                      in_=ps_t)

            # link colsum partial (this core's 256 rows), via ones matmuls
            for ch in range(4):
                ps_cs = psG.tile([1, 512], dt, tag="tr", bufs=2)
                for i in range(2):
                    nc.tensor.matmul(ps_cs, lhsT=ones_cb,
                                     rhs=lt2[:, i, ch * 512:(ch + 1) * 512],
                                     start=(i == 0), stop=(i == 1))
                nc.scalar.copy(out=csrow[:, ch * 512:(ch + 1) * 512],
                               in_=ps_cs)

        # ---- link rowsum shard + allocation shard (DVE/scalar only) ----
        for i in range(2):
            rcol = wk.tile([128, 1], dt, tag="rcol", bufs=2)
            nc.vector.tensor_reduce(out=rcol, in_=lt2[:, i, :],
                                    axis=mybir.AxisListType.X, op=ALU.add)
            nc.vector.tensor_copy(out=rs2[:, i:i + 1], in_=rcol)
        L_b = wk.tile([128, N], dt, tag="big2")
        nc.scalar.activation(out=L_b, in_=usg_b, func=AF.Ln, bias=1.0,
                             scale=-1.0)
        for i in range(2):
            u_col = uc_sb[:, i:i + 1]
            step = wk.tile([128, N], dt, tag="big3")
            nc.vector.tensor_scalar(out=step, in0=usg_b, scalar1=u_col,
                                    scalar2=None, op0=ALU.is_lt)
            nc.vector.tensor_tensor(out=step, in0=step, in1=L_b, op=ALU.mult)
            a_col = wk.tile([128, 1], dt, tag="acol", bufs=2)
            nc.vector.tensor_reduce(out=a_col, in_=step,
                                    axis=mybir.AxisListType.X, op=ALU.add)
            nc.scalar.activation(out=a_col, in_=a_col, func=AF.Exp)
            nc.vector.tensor_mul(out=a_col, in0=a_col, in1=u_col)
            nc.vector.tensor_copy(out=al2[:, i:i + 1], in_=a_col)

        # ======== phase 2 (ppb/ppt/pp1 psum scope): rest of kernel ========
        ppb = es.enter_context(tc.tile_pool(name="ppb", bufs=1, space="PSUM"))
        ppt = es.enter_context(tc.tile_pool(name="ppt", bufs=2, space="PSUM"))
        pp1 = es.enter_context(tc.tile_pool(name="pp1", bufs=1, space="PSUM"))

        # interface vector for own 32 rows
        ps_itf = ppb.tile([BL, IF], dt, tag="big")
        for k in range(KH):
            nc.tensor.matmul(ps_itf[:, 0:512], lhsT=hT[:, k, :],
                             rhs=wif_t[k][:, 0:512], start=(k == 0),
                             stop=False)
            nc.tensor.matmul(ps_itf[:, 512:IF], lhsT=hT[:, k, :],
                             rhs=wif_t[k][:, 512:IF], start=(k == 0),
                             stop=False)
        nc.tensor.matmul(ps_itf[:, 0:512], lhsT=ones1b,
                         rhs=bif_sb[0:1, 0:512], start=False, stop=True)
        nc.tensor.matmul(ps_itf[:, 512:IF], lhsT=ones1b,
                         rhs=bif_sb[0:1, 512:IF], start=False, stop=True)
        itf = wk.tile([BL, IF], dt)
        nc.vector.tensor_copy(out=itf, in_=ps_itf)

        # output-projection h-part, accumulated now (off critical path)
        ps_fh = pp1.tile([BL, 1024], dt, tag="p1")
        for k in range(KH):
            for hf in range(2):
                nc.tensor.matmul(
                    ps_fh[:, hf * 512:(hf + 1) * 512], lhsT=hT[:, k, :],
                    rhs=woutsb[:, k * 1024 + hf * 512:k * 1024 + (hf + 1) * 512],
                    start=(k == 0), stop=False)
        for hf in range(2):   # + b_out via rank-1 ones x bias row
            nc.tensor.matmul(ps_fh[:, hf * 512:(hf + 1) * 512], lhsT=ones1b,
                             rhs=bout_sb[0:1, hf * 512:(hf + 1) * 512],
                             start=False, stop=True)
        outacc = cons.tile([BL, 1024], dt)
        nc.vector.tensor_copy(out=outacc, in_=ps_fh)

        # ---- write addressing factors -> X payload row [32, 388] ----
        xrow = wk.tile([BL, XW], BF16)
        wv = itf[:, 0:128]
        nc.scalar.activation(out=xrow[:, 128:256], in_=itf[:, 128:256],
                             func=AF.Sigmoid)    # sig(erase), *wg below
        wgag = wk.tile([BL, 2], dt)
        nc.scalar.activation(out=wgag, in_=itf[:, 256:258], func=AF.Sigmoid)
        wg = wgag[:, 0:1]
        agt = wgag[:, 1:2]
        expm = wk.tile([BL, 12], dt)
        nc.scalar.activation(out=expm, in_=itf[:, 259:271], func=AF.Exp)
        msum = wk.tile([BL, 4], dt)
        nc.vector.tensor_reduce(out=msum,
                                in_=expm.rearrange("p (r k) -> p r k", k=3),
                                axis=mybir.AxisListType.X, op=ALU.add)
        minv = wk.tile([BL, 4], dt)
        nc.vector.reciprocal(out=minv, in_=msum)
        sc16 = wk.tile([BL, 16], dt)   # [rstr | m0 | m1 | m2]
        nc.scalar.activation(out=sc16[:, 0:4], in_=itf[:, 271:275],
                             func=AF.Exp)
        nc.scalar.activation(out=sc16[:, 0:4], in_=sc16[:, 0:4],
                             func=AF.Ln, bias=1.0)
        em3 = expm.rearrange("p (r k) -> p r k", k=3)
        for kk in range(3):
            nc.vector.tensor_mul(out=sc16[:, 4 + 4 * kk:8 + 4 * kk],
                                 in0=em3[:, :, kk], in1=minv)
        ps_t16 = ppt.tile([16, BL], dt, tag="tr")
        nc.tensor.transpose(ps_t16, sc16, ident[0:BL, 0:BL])
        t16 = wk.tile([16, BL], dt)
        nc.vector.tensor_copy(out=t16, in_=ps_t16)
        cols4 = wk.tile([128, 4], dt)  # [str | m0 | m1 | m2] as rb-columns
        for q in range(4):
            nc.sync.dma_start(out=cols4[:, q:q + 1],
                              in_=t16[4 * q:4 * q + 4, :])
        str_col = cols4[:, 0:1]
        m0_col = cols4[:, 1:2]

        nc.vector.tensor_scalar(out=xrow[:, 128:256], in0=xrow[:, 128:256],
                                scalar1=wg, scalar2=None, op0=ALU.mult)
        nc.vector.tensor_scalar(out=xrow[:, 256:384], in0=wv, scalar1=wg,
                                scalar2=None, op0=ALU.mult)
        nc.vector.tensor_copy(out=xrow[:, 384:385], in_=wg)
        b_half = wk.tile([BL, 1], dt)
        nc.vector.tensor_mul(out=b_half, in0=wg, in1=agt)
        nc.vector.tensor_scalar(out=xrow[:, 385:386], in0=b_half, scalar1=0.5,
                                scalar2=None, op0=ALU.mult)

        sq = wk.tile([BL, 128], dt, tag="sq")
        nrm = wk.tile([BL, 1], dt, tag="nrm")
        nc.scalar.activation(out=sq, in_=wv, func=AF.Square, accum_out=nrm)
        nc.scalar.activation(out=nrm, in_=nrm, func=AF.Sqrt)
        nc.vector.tensor_scalar(out=nrm, in0=nrm, scalar1=1e-12, scalar2=None,
                                op0=ALU.max)
        nc.vector.reciprocal(out=nrm, in_=nrm)
        nc.vector.tensor_scalar(out=xrow[:, 0:128], in0=wv, scalar1=nrm,
                                scalar2=None, op0=ALU.mult)

        # read-key norms + mode-column transposes: no AG dependency, do now
        nkT = wk.tile([128, 128], BF16)
        rk3 = itf[:, 275:787].rearrange("p (r d) -> p r d", d=128)
        sqk3 = wk.tile([BL, R, 128], dt)
        nc.vector.tensor_mul(out=sqk3, in0=rk3, in1=rk3)
        nrk4 = wk.tile([BL, R], dt)
        nc.vector.tensor_reduce(out=nrk4, in_=sqk3, axis=mybir.AxisListType.X,
                                op=ALU.add)
        nc.scalar.activation(out=nrk4, in_=nrk4, func=AF.Sqrt)
        nc.vector.tensor_scalar(out=nrk4, in0=nrk4, scalar1=1e-12,
                                scalar2=None, op0=ALU.max)
        nc.vector.reciprocal(out=nrk4, in_=nrk4)
        sqk3b = wk.tile([BL, R, 128], BF16)
        nc.vector.tensor_tensor(
            out=sqk3b, in0=rk3,
            in1=nrk4.unsqueeze(2).broadcast_to([BL, R, 128]), op=ALU.mult)
        for r in range(R):
            ps_k = ppt.tile([128, BL], BF16, tag="tr")
            nc.tensor.transpose(ps_k, sqk3b[:, r, :], ident_bf[0:BL, 0:BL])
            nc.vector.tensor_copy(out=nkT[:, r * BL:(r + 1) * BL], in_=ps_k)
        ps_m1 = ppt.tile([1, 128], dt, tag="tr")
        nc.tensor.transpose(ps_m1, cols4[:, 2:3], ident)
        m1T = wk.tile([1, 128], BF16)
        nc.vector.tensor_copy(out=m1T, in_=ps_m1)
        ps_m2 = ppt.tile([1, 128], dt, tag="tr")
        nc.tensor.transpose(ps_m2, cols4[:, 3:4], ident)
        m2T = wk.tile([1, 128], BF16)
        nc.vector.tensor_copy(out=m2T, in_=ps_m2)

        # ---- payload stores + the single AllGather ----
        nc.sync.dma_start(
            out=ag_in[P_X:P_X + BL * XW].rearrange("(p f) -> p f", p=BL),
            in_=xrow)
        nc.scalar.dma_start(
            out=ag_in[P_AL:P_AL + 256].rearrange("(t p) -> p t", p=128),
            in_=al2)
        nc.scalar.dma_start(
            out=ag_in[P_RS:P_RS + 256].rearrange("(t p) -> p t", p=128),
            in_=rs2)
        nc.sync.dma_start(out=ag_in[P_CS:P_CS + N], in_=csrow)

        nc.gpsimd.collective_compute(
            "AllGather", ALU.bypass, replica_groups=[list(range(M))],
            ins=[ag_in[:]], outs=[ag_out.flatten()])

        # ---- post-AG loads (few, fat, split across queues) ----
        xf = []
        for bc in range(2):
            t_x = wk.tile([128, XW], BF16, name=f"xf{bc}")
            for r in range(4):
                rk = bc * 4 + r
                eng = nc.sync if r % 2 == 0 else nc.scalar
                eng.dma_start(
                    out=t_x[r * BL:(r + 1) * BL, :],
                    in_=ag_out[rk, P_X:P_X + BL * XW]
                    .rearrange("(p f) -> p f", p=BL))
            xf.append(t_x)
        alrs = wk.tile([1, M * 512], BF16)
        for r in range(M):
            eng = nc.sync if r % 2 == 0 else nc.scalar
            eng.dma_start(out=alrs[0:1, r * 512:(r + 1) * 512],
                          in_=ag_out[r, P_AL:P_AL + 512])
        cs8 = wk.tile([M, N], BF16)
        nc.scalar.dma_start(out=cs8, in_=ag_out[:, P_CS:P_CS + N])

        # content-weight softmax for all 256 rows (redundant on every core)
        nwvT = []
        cwf = []
        den2 = wk.tile([128, 2], dt)
        for bc in range(2):
            ps_nt = ppt.tile([128, 128], BF16, tag="tr")
            nc.tensor.transpose(ps_nt, xf[bc][:, 0:128], ident_bf)
            t_nt = wk.tile([128, 128], BF16, name=f"nwvT{bc}")
            nc.vector.tensor_copy(out=t_nt, in_=ps_nt)
            nwvT.append(t_nt)
        for bc in range(2):
            ps_cw = ppb.tile([128, N], dt, tag="big")
            for ch in range(4):
                nc.tensor.matmul(ps_cw[:, ch * 512:(ch + 1) * 512],
                                 lhsT=nwvT[bc],
                                 rhs=memnT[:, ch * 512:(ch + 1) * 512],
                                 start=True, stop=True)
            t_cw = wk.tile([128, N], BF16, name=f"cwf{bc}")
            nc.scalar.activation(out=t_cw, in_=ps_cw, func=AF.Exp,
                                 accum_out=den2[:, bc:bc + 1])
            cwf.append(t_cw)
        # a_sc = 0.5*wg/den ; rhs rows [a_sc*ev | a_sc*av]
        nc.vector.reciprocal(out=den2, in_=den2)
        asc2 = wk.tile([128, 2], dt)
        for bc in range(2):
            nc.vector.tensor_mul(out=asc2[:, bc:bc + 1],
                                 in0=xf[bc][:, 384:385],
                                 in1=den2[:, bc:bc + 1])
        nc.vector.tensor_scalar(out=asc2, in0=asc2, scalar1=0.5, scalar2=None,
                                op0=ALU.mult)
        rhs_eva = wk.tile([128, 2, 256], BF16)
        for bc in range(2):
            nc.vector.tensor_scalar(out=rhs_eva[:, bc, :],
                                    in0=xf[bc][:, 128:384],
                                    scalar1=asc2[:, bc:bc + 1],
                                    scalar2=None, op0=ALU.mult)

        # evb/avb row = sum_b b_half[b]*[ev|av][b,:]  (rank-1 outer factor)
        ps_evb = ppt.tile([1, 256], dt, tag="tr")
        for bc in range(2):
            nc.tensor.matmul(ps_evb, lhsT=xf[bc][:, 385:386],
                             rhs=xf[bc][:, 128:384], start=(bc == 0),
                             stop=(bc == 1))
        evab_r = wk.tile([1, 256], BF16)
        nc.vector.tensor_copy(out=evab_r, in_=ps_evb)

        # bw/fw rows: 0.9/N * [colsum(link), rowsum(link)]
        bwrow = wk.tile([1, N], BF16)
        for ch in range(4):
            ps_cs2 = ppt.tile([1, 512], dt, tag="tr")
            nc.tensor.matmul(ps_cs2, lhsT=ones_cb[0:8, :],
                             rhs=cs8[:, ch * 512:(ch + 1) * 512],
                             start=True, stop=True)
            nc.vector.tensor_scalar(out=bwrow[:, ch * 512:(ch + 1) * 512],
                                    in0=ps_cs2, scalar1=0.9 / N,
                                    scalar2=None, op0=ALU.mult)
        fwrow = wk.tile([1, N], BF16)
        for r in range(M):
            nc.vector.tensor_scalar(
                out=fwrow[:, r * 256:(r + 1) * 256],
                in0=alrs[:, r * 512 + 256:r * 512 + 512],
                scalar1=0.9 / N, scalar2=None, op0=ALU.mult)

        # ---- erase/add matmuls + mnew, pipelined per n-tile ----
        mnew = wk.tile([128, N], dt, tag="big1")
        mnew3 = mnew.rearrange("p (t d) -> p t d", d=128)
        for t in range(NT):
            ps_ea = ppt.tile([128, 256], dt, tag="tr")
            al_sl = alrs[0:1, 512 * (t // 2) + 128 * (t % 2):
                          512 * (t // 2) + 128 * (t % 2) + 128]
            for bc in range(2):
                nc.tensor.matmul(ps_ea, lhsT=cwf[bc][:, t * 128:(t + 1) * 128],
                                 rhs=rhs_eva[:, bc, :], start=(bc == 0),
                                 stop=False)
            nc.tensor.matmul(ps_ea, lhsT=al_sl, rhs=evab_r, start=False,
                             stop=True)
            f1 = wk.tile([128, 128], dt, tag="f1", bufs=2)
            nc.vector.tensor_scalar(out=f1, in0=ps_ea[:, 0:128],
                                    scalar1=-1.0 / B, scalar2=1.0,
                                    op0=ALU.mult, op1=ALU.add)
            nc.vector.tensor_mul(out=f1, in0=f1, in1=mem3[:, t, :])
            nc.vector.scalar_tensor_tensor(out=mnew3[:, t, :],
                                           in0=ps_ea[:, 128:256],
                                           scalar=1.0 / B, in1=f1,
                                           op0=ALU.mult, op1=ALU.add)

        # mnew row norms + normalized transpose, in halves so the DVE norm
        # chain of half 1 overlaps TensorE transposes of half 0
        HT2 = NT // 2
        sqf = wk.tile([128, NT, 128], dt, tag="big2")
        nrm16 = wk.tile([128, NT], dt)
        nmn = wk.tile([128, NT, 128], BF16, tag="bigb1")
        mnewT = memnT  # reuse buffer: memnT dead after cw sim
        for hh in range(2):
            sl = slice(hh * HT2, (hh + 1) * HT2)
            nc.vector.tensor_mul(out=sqf[:, sl, :], in0=mnew3[:, sl, :],
                                 in1=mnew3[:, sl, :])
            nc.vector.tensor_reduce(out=nrm16[:, sl], in_=sqf[:, sl, :],
                                    axis=mybir.AxisListType.X, op=ALU.add)
            nc.scalar.activation(out=nrm16[:, sl], in_=nrm16[:, sl],
                                 func=AF.Sqrt)
            nc.vector.tensor_scalar(out=nrm16[:, sl], in0=nrm16[:, sl],
                                    scalar1=1e-12, scalar2=None, op0=ALU.max)
            nc.vector.reciprocal(out=nrm16[:, sl], in_=nrm16[:, sl])
            nc.vector.tensor_tensor(
                out=nmn[:, sl, :], in0=mnew3[:, sl, :],
                in1=nrm16[:, sl].unsqueeze(2).broadcast_to([128, HT2, 128]),
                op=ALU.mult)
            for t in range(hh * HT2, (hh + 1) * HT2):
                ps_t = ppt.tile([128, 128], BF16, tag="tr")
                nc.tensor.transpose(ps_t, nmn[:, t, :], ident_bf)
                nc.scalar.copy(out=mnewT[:, t * 128:(t + 1) * 128], in_=ps_t)
        mnew_bf = wk.tile([128, N], BF16, tag="bigb2")
        nc.vector.tensor_copy(out=mnew_bf, in_=mnew)

        ps_sim = ppb.tile([128, N], dt, tag="big")
        for ch in range(4):
            nc.tensor.matmul(ps_sim[:, ch * 512:(ch + 1) * 512], lhsT=nkT,
                             rhs=mnewT[:, ch * 512:(ch + 1) * 512],
                             start=True, stop=True)
        esim = wk.tile([128, N], dt, tag="big2")
        dsum = wk.tile([128, 1], dt)
        nc.scalar.activation(out=esim, in_=ps_sim, func=AF.Exp, scale=str_col,
                             accum_out=dsum)
        nc.vector.reciprocal(out=dsum, in_=dsum)
        c0 = wk.tile([128, 1], dt)
        nc.vector.tensor_mul(out=c0, in0=m0_col, in1=dsum)
        ps_term = ppb.tile([128, N], dt, tag="big")
        for ch in range(4):
            nc.tensor.matmul(ps_term[:, ch * 512:(ch + 1) * 512], lhsT=m1T,
                             rhs=bwrow[:, ch * 512:(ch + 1) * 512],
                             start=True, stop=False)
            nc.tensor.matmul(ps_term[:, ch * 512:(ch + 1) * 512], lhsT=m2T,
                             rhs=fwrow[:, ch * 512:(ch + 1) * 512],
                             start=False, stop=True)
        nrw = esim
        for ch in range(4):
            nc.vector.scalar_tensor_tensor(
                out=nrw[:, ch * 512:(ch + 1) * 512],
                in0=esim[:, ch * 512:(ch + 1) * 512], scalar=c0,
                in1=ps_term[:, ch * 512:(ch + 1) * 512], op0=ALU.mult,
                op1=ALU.add)

        ps_ro = pp1.tile([128, 128], dt, tag="p1")
        roT = wk.tile([128, 128], BF16)
        for t in range(NT):
            ps_tr = ppt.tile([128, 128], dt, tag="tr")
            nc.tensor.transpose(ps_tr, nrw[:, t * 128:(t + 1) * 128], ident)
            nrwT = wk.tile([128, 128], BF16, tag="nrwT", bufs=2)
            nc.scalar.copy(out=nrwT, in_=ps_tr)
            nc.tensor.matmul(ps_ro, lhsT=mnew_bf[:, t * 128:(t + 1) * 128],
                             rhs=nrwT, start=(t == 0), stop=(t == NT - 1))
        nc.vector.tensor_copy(out=roT, in_=ps_ro)

        # ---- output projection read-part + writeback ----
        ps_f2 = pp1.tile([BL, 1024], dt, tag="p1")
        for k in range(4):
            for hf in range(2):
                nc.tensor.matmul(
                    ps_f2[:, hf * 512:(hf + 1) * 512],
                    lhsT=roT[:, k * BL:(k + 1) * BL],
                    rhs=woutsb[:, (KH + k) * 1024 + hf * 512:
                               (KH + k) * 1024 + (hf + 1) * 512],
                    start=(k == 0), stop=(k == 3))
        nc.vector.tensor_tensor(out=outacc, in0=ps_f2, in1=outacc, op=ALU.add)
        nc.sync.dma_start(out=outF[:, :], in_=outacc)

        if DEBUG:
            nc.sync.dma_start(out=d_h[:, :], in_=hrow)
            nc.sync.dma_start(out=d_itf[:, :], in_=itf)
            d_cwf = wk.tile([128, N], dt)
            nc.vector.tensor_copy(out=d_cwf, in_=cwf[0])
            nc.sync.dma_start(out=d_cw[:, :], in_=d_cwf)
            nc.sync.dma_start(out=d_mnew[:, :], in_=mnew)
            nc.sync.dma_start(out=d_nrw[:, :], in_=nrw)
            nc.sync.dma_start(out=d_bw[0:1, :], in_=bwrow)
            nc.sync.dma_start(out=d_bw[1:2, :], in_=fwrow)

    nc.finalize()
    return nc


def _prep_inputs(x, memory, usage, link, W_ih, W_hh, b_ih, b_hh, W_if, b_if,
                 W_out, b_out):
    f = np.float32
    x = np.asarray(x, f); memory = np.asarray(memory, f)
    usage = np.asarray(usage, f); link = np.asarray(link, f)
    W_ih = np.asarray(W_ih, f); b_ih = np.asarray(b_ih, f)
    b_hh = np.asarray(b_hh, f); W_if = np.asarray(W_if, f)
    b_if = np.asarray(b_if, f); W_out = np.asarray(W_out, f)
    b_out = np.asarray(b_out, f)

    sel = np.r_[0:1024, 2048:4096]
    W3T = W_ih[sel].T                             # (1536, 3072)
    w3r = np.ascontiguousarray(
        W3T.reshape(KC, 128, 3072).transpose(1, 0, 2)
        .reshape(128, KC * 3072).astype(ml_dtypes.bfloat16))
    b3row = np.ascontiguousarray(
        (b_ih + b_hh)[sel].reshape(1, 3072).astype(ml_dtypes.bfloat16))
    wifr = np.ascontiguousarray(
        W_if.T.reshape(KH, 128, IF).transpose(1, 0, 2)
        .reshape(128, KH * IF).astype(ml_dtypes.bfloat16))
    woutr = np.ascontiguousarray(
        W_out.T.reshape(KC, 128, 1024).transpose(1, 0, 2)
        .reshape(128, KC * 1024).astype(ml_dtypes.bfloat16))
    boutr = b_out.reshape(1, 1024).astype(ml_dtypes.bfloat16)
    memA = np.ascontiguousarray(
        memory.reshape(NT, 128, 128).transpose(1, 0, 2).reshape(128, N))
    bifr = b_if.reshape(1, IF).astype(ml_dtypes.bfloat16)
    usgr = usage.reshape(1, N)

    shared = dict(w3r=w3r, b3row=b3row, wifr=wifr, bifr=bifr, woutr=woutr,
                  boutr=boutr, memA=memA, usg=usgr)
    in_maps = []
    for c in range(M):
        xs = x[c * BL:(c + 1) * BL]               # (32, 1024)
        cinx = np.ascontiguousarray(
            xs.T.reshape(KH, 128, BL).transpose(1, 0, 2)
            .reshape(128, KH * BL).astype(ml_dtypes.bfloat16))
        ls = link[c * 256:(c + 1) * 256]          # (256, 2048)
        lnkm = np.ascontiguousarray(
            ls.reshape(2, 128, N).transpose(1, 0, 2)
            .reshape(128, 2 * N).astype(ml_dtypes.bfloat16))
        ucols = np.ascontiguousarray(
            usage.reshape(NT, 128)[2 * c:2 * c + 2].T)      # (128, 2)
        m = dict(shared)
        m["cinx"] = cinx
        m["lnk"] = lnkm
        m["ucols"] = ucols
        in_maps.append(m)
    return in_maps


def kernel(**inputs):
    nc = build_nc()
    in_maps = _prep_inputs(**inputs)
    res = run_bass_kernel_spmd(nc, in_maps, list(range(M))).results
    return np.concatenate([res[c]["outF"] for c in range(M)],
                          0).astype(np.float32)
